# revision 4
# baseline (speedup 1.0000x reference)
"""Trainium2 Bass kernel for the 2-layer hyperbolic (Poincare ball) GCN encoder.

Strategy (8 NeuronCores, SPMD):
  - Nodes sharded across cores (2500 rows/core, padded to 2560 = 20 tiles of 128),
    with a per-core degree-balanced permutation so every 128-destination block
    has ~equal edge count.
  - Weights replicated (bf16); dense mobius_matvec/mobius_add/logmap0 computed on
    the owned shard with all per-row reductions fused into per-partition scalar
    "grid" tensors of shape [128, T].
  - Per-layer exchange: tangent features (pre-scaled by deg^-0.5 on the source
    side) are AllGathered in bf16 across the 8 cores, SPLIT INTO TWO HALF-SHARD
    COLLECTIVES so each half is triggered as soon as its stageA tiles finish.
    A tiny warmup AllGather at kernel start absorbs the one-time CC-library
    load / mesh setup (~150us on the profiled baseline).
  - Edges partitioned by destination and CLASSIFIED BY SOURCE HALF: within each
    128-destination block, chunks whose sources live in half 0 are processed
    first, so phaseB's gathers begin while the half-1 AllGather is in flight.
    Per-block per-class chunk counts are ragged (baked at build time).
  - Messages fetched with dma_gather (1024 rows per instruction) round-robined
    over 4 SWDGE queues with rotating msg buffers; segment-sum on TensorE via
    0/1 selection matrices accumulated in PSUM.
  - Layer-1 phaseA is emitted inside layer-0 phaseB (pass1 in the group
    epilogues; stageA half 0 + its AllGather right after block T/2-1), so the
    layer-1 exchange is fully hidden behind layer-0's gather/matmul pipeline.
"""
import os
import numpy as np
import ml_dtypes

import concourse.bass as bass
import concourse.bacc as bacc
import concourse.tile as tile
import concourse.mybir as mybir
from concourse.bass_utils import run_bass_kernel_spmd
from concourse.masks import make_identity

NCORES = 8
P = 128
GN = 1024            # indices per dma_gather
CPG = GN // P        # chunks per gather
NQ = 4               # SWDGE queues
MN = 1.0 - 4e-3
EPS = 1e-15
ATEPS = 1e-7

f32 = mybir.dt.float32
bf16 = mybir.dt.bfloat16
i16 = mybir.dt.int16
AF = mybir.ActivationFunctionType
OP = mybir.AluOpType

_prog_cache = {}


# ----------------------------------------------------------------- host side

def _np_expmap0(u):
    u = np.asarray(u, np.float32)
    n = max(float(np.linalg.norm(u)), EPS)
    v = (np.tanh(n) * u / n).astype(np.float32)
    nn = max(float(np.linalg.norm(v)), EPS)
    if nn > MN:
        v = (v / nn * MN).astype(np.float32)
    return v


def _wrap_idx(lin, NGs):
    """[NGs*GN] linear slot order -> int16 [128, NGs*(GN//16)] wrapped."""
    w = lin.reshape(NGs, GN // 16, 16).transpose(2, 0, 1).reshape(16, -1)
    return np.tile(w.astype(np.int16), (8, 1))


def _host_prep(x, edge_index):
    x = np.asarray(x, np.float32)
    ei = np.asarray(edge_index)
    N, D = x.shape
    assert N % NCORES == 0
    n_loc = N // NCORES
    T = (n_loc + P - 1) // P
    assert T % 2 == 0
    TH = T // 2
    HR = TH * P                      # rows per half per core
    n_pad = T * P
    assert NCORES * HR <= 32767, "indices must fit int16"

    loops = np.arange(N, dtype=ei.dtype)
    ei = np.concatenate([ei, np.stack([loops, loops])], axis=1)
    row, col = ei[0].astype(np.int64), ei[1].astype(np.int64)
    deg = np.bincount(col, minlength=N).astype(np.float32)
    dis = (deg ** -0.5).astype(np.float32)

    # --- per-core degree-balanced slot permutation -------------------------
    inv_perm = np.empty((NCORES, n_loc), np.int64)   # local node -> slot
    perm = np.full((NCORES, n_pad), -1, np.int64)    # slot -> local node
    for r in range(NCORES):
        dloc = deg[r * n_loc:(r + 1) * n_loc]
        order = np.argsort(-dloc, kind="stable")
        ids = np.full(n_pad, -1, np.int64)
        ids[:n_loc] = order
        ids = ids.reshape(P, T)
        ids[1::2] = ids[1::2, ::-1]
        for b in range(T):
            blk_nodes = ids[:, b]
            for j, nd in enumerate(blk_nodes):
                slot = b * P + j
                perm[r, slot] = nd
                if nd >= 0:
                    inv_perm[r, nd] = slot

    src_core = row // n_loc
    src_slot = inv_perm[src_core, row % n_loc]
    src_half = src_slot // HR                         # 0 or 1
    src_row = src_core * HR + (src_slot % HR)         # row in ts_full[half]
    dst_core = col // n_loc
    dst_slot = inv_perm[dst_core, col % n_loc]

    blk = dst_slot // P + dst_core * T
    order = np.lexsort((src_row, src_half, blk))
    src_s = src_row[order]
    half_s = src_half[order]
    blk_s = blk[order]
    dstrel_s = (dst_slot[order] % P).astype(np.float32)

    blk_counts = np.bincount(blk_s, minlength=NCORES * T)
    bounds = np.concatenate([[0], np.cumsum(blk_counts)])

    # Ragged per-block per-class chunk layout. Identical across cores in
    # CHUNK COUNTS is required (one SPMD program) -> use the max per (b,cls)
    # over cores and pad. Slot streams are per-core.
    c_cnt = np.zeros((NCORES, T, 2), np.int64)
    for r in range(NCORES):
        for b in range(T):
            lo, hi = bounds[r * T + b], bounds[r * T + b + 1]
            h = half_s[lo:hi]
            n0 = int((h == 0).sum())
            n1 = int(hi - lo - n0)
            c_cnt[r, b, 0] = (n0 + P - 1) // P
            c_cnt[r, b, 1] = (n1 + P - 1) // P
    cmax = c_cnt.max(axis=0)                 # [T, 2] chunks per block/class
    c0 = [int(v) for v in cmax[:, 0]]
    c1 = [int(v) for v in cmax[:, 1]]
    tot0, tot1 = sum(c0), sum(c1)
    NG0 = (tot0 * P + GN - 1) // GN
    NG1 = (tot1 * P + GN - 1) // GN

    # per-core slot streams (gather indices + dst-rel for S build)
    gl0 = np.zeros((NCORES, NG0 * GN), np.int64)
    gl1 = np.zeros((NCORES, NG1 * GN), np.int64)
    edst = np.full((NCORES, P, tot0 + tot1), -1.0, np.float32)
    off0 = np.concatenate([[0], np.cumsum(c0)])
    off1 = np.concatenate([[0], np.cumsum(c1)])
    colof = np.concatenate([[0], np.cumsum([a + b for a, b in zip(c0, c1)])])
    for r in range(NCORES):
        for b in range(T):
            lo, hi = bounds[r * T + b], bounds[r * T + b + 1]
            h = half_s[lo:hi]
            for cls, (glin, offs) in enumerate(((gl0, off0), (gl1, off1))):
                sel = np.nonzero(h == cls)[0] + lo
                L = len(sel)
                nch = (c0[b] if cls == 0 else c1[b])
                pad = np.zeros(nch * P, np.int64)
                pad[:L] = src_s[sel]
                glin[r, offs[b] * P:(offs[b] + nch) * P] = pad
                dpad = np.full(nch * P, -1.0, np.float32)
                dpad[:L] = dstrel_s[sel]
                cbase = colof[b] + (0 if cls == 0 else c0[b])
                edst[r][:, cbase:cbase + nch] = dpad.reshape(nch, P).T

    idx0 = np.stack([_wrap_idx(gl0[r], NG0) for r in range(NCORES)])
    idx1 = np.stack([_wrap_idx(gl1[r], NG1) for r in range(NCORES)])

    dis_loc = np.zeros((NCORES, P, T), np.float32)
    x_loc = np.zeros((NCORES, n_pad, D), np.float32)
    for r in range(NCORES):
        for slot in range(n_pad):
            nd = perm[r, slot]
            if nd >= 0:
                dis_loc[r, slot % P, slot // P] = dis[r * n_loc + nd]
                x_loc[r, slot] = x[r * n_loc + nd]

    iota = np.tile(np.arange(P, dtype=np.float32)[None, :], (P, 1))
    meta = dict(N=N, D=D, n_loc=n_loc, T=T, n_pad=n_pad, perm=perm,
                c0=c0, c1=c1, NG0=NG0, NG1=NG1)
    per_core = [dict(x=x_loc[r].astype(ml_dtypes.bfloat16),
                     dis=dis_loc[r],
                     gidx0=idx0[r], gidx1=idx1[r],
                     edst=edst[r].astype(ml_dtypes.bfloat16),
                     iota=iota.astype(ml_dtypes.bfloat16))
                for r in range(NCORES)]
    return meta, per_core


# --------------------------------------------------------------- device side

def _build_program(T, DC, c0, c1, NG0, NG1):
    D = DC * P
    TH = T // 2
    HR = TH * P
    NPAD = T * P
    EX = bf16
    TOT = sum(c0) + sum(c1)
    CMAX = max(a + b for a, b in zip(c0, c1))
    GRP = int(os.environ.get("KGRP", "5"))
    MSGB = int(os.environ.get("KMSGB", "7"))

    nc = bacc.Bacc("TRN2", target_bir_lowering=False, debug=False,
                   num_devices=NCORES, num_swdge_queues=NQ,
                   dynamic_dma_scratch_size=int(os.environ.get("KSCRATCH",
                                                               "24576")))

    x_in = nc.dram_tensor("x", [NPAD, D], bf16, kind="ExternalInput")
    wt_in = nc.dram_tensor("wt", [2, D, D], bf16, kind="ExternalInput")
    y_in = nc.dram_tensor("y", [2, P, D], f32, kind="ExternalInput")
    iota_in = nc.dram_tensor("iota", [P, P], bf16, kind="ExternalInput")
    dis_in = nc.dram_tensor("dis", [P, T], f32, kind="ExternalInput")
    g0_in = nc.dram_tensor("gidx0", [P, NG0 * (GN // 16)], i16,
                           kind="ExternalInput")
    g1_in = nc.dram_tensor("gidx1", [P, NG1 * (GN // 16)], i16,
                           kind="ExternalInput")
    edst_in = nc.dram_tensor("edst", [P, TOT], bf16, kind="ExternalInput")
    out_ext = nc.dram_tensor("out", [NPAD, D], f32, kind="ExternalOutput")

    with tile.TileContext(nc) as tc:
        with (
            tc.tile_pool(name="const", bufs=1) as constp,
            tc.tile_pool(name="grid", bufs=1) as gridp,
            tc.tile_pool(name="big", bufs=1) as bigp,
            tc.tile_pool(name="work", bufs=3) as workp,
            tc.tile_pool(name="junk", bufs=3) as junkp,
            tc.tile_pool(name="msgs", bufs=MSGB) as msgp,
            tc.tile_pool(name="sblk", bufs=2) as sblkp,
            tc.tile_pool(name="psum", bufs=2, space="PSUM") as psump,
            tc.tile_pool(name="psag", bufs=3, space="PSUM") as psagp,
            tc.tile_pool(name="dram", bufs=1, space="DRAM") as dramp,
        ):
            # ---- CC warmup: tiny AllGather issued before anything else ----
            warm_l = dramp.tile([P, 16], bf16, name="warm_l", tag="warm_l")
            warm_g = dramp.tile([NCORES * P, 16], bf16, addr_space="Shared",
                                name="warm_g", tag="warm_g")
            nc.sync.dma_start(out=warm_l[:], in_=iota_in[:, 0:16])
            nc.gpsimd.collective_compute(
                "AllGather", OP.bypass,
                replica_groups=[list(range(NCORES))],
                ins=[warm_l.opt()], outs=[warm_g.opt()])

            # ---- constants ----
            wt_sb = constp.tile([P, 2 * DC * D], bf16, name="wt", tag="wt")
            for l in range(2):
                for k in range(DC):
                    nc.sync.dma_start(
                        out=wt_sb[:, (l * DC + k) * D:(l * DC + k + 1) * D],
                        in_=wt_in[l, k * P:(k + 1) * P, :])
            y_sb = constp.tile([P, 2 * D], f32, name="y", tag="y")
            nc.sync.dma_start(out=y_sb[:, 0:D], in_=y_in[0])
            nc.sync.dma_start(out=y_sb[:, D:2 * D], in_=y_in[1])
            iota_sb = constp.tile([P, P], bf16, name="iota", tag="iota")
            nc.sync.dma_start(out=iota_sb[:], in_=iota_in[:, :])
            ident = constp.tile([P, P], f32, name="ident", tag="ident")
            make_identity(nc, ident[:])
            disg = constp.tile([P, T], f32, name="dis", tag="dis")
            nc.sync.dma_start(out=disg[:], in_=dis_in[:, :])
            g0_sb = constp.tile([P, NG0 * (GN // 16)], i16, name="g0",
                                tag="g0")
            nc.sync.dma_start(out=g0_sb[:], in_=g0_in[:, :])
            g1_sb = constp.tile([P, NG1 * (GN // 16)], i16, name="g1",
                                tag="g1")
            nc.sync.dma_start(out=g1_sb[:], in_=g1_in[:, :])
            edst_sb = constp.tile([P, TOT], bf16, name="edst", tag="edst")
            nc.sync.dma_start(out=edst_sb[:], in_=edst_in[:, :])

            # ---- persistent big tensors ----
            h_grid = bigp.tile([P, T * D], f32, name="h", tag="h")  # h then u
            agg_grid = bigp.tile([P, T * D], bf16, name="agg", tag="agg")
            xstage = bigp.tile([P, TH * D], bf16, name="xs", tag="xs")
            hn2 = gridp.tile([P, T], f32, name="hn2", tag="hn2")

            def G(tag):
                return gridp.tile([P, T], f32, name=tag, tag=tag)

            def tsl(t):
                return slice(t * D, (t + 1) * D)

            mxn2_g = [G("mxn2_0"), G("mxn2_1")]

            def emit_pass1(l, t):
                pt = psump.tile([P, D], f32, name="pt", tag="pt")
                for k in range(DC):
                    nc.tensor.transpose(
                        out=pt[:, k * P:(k + 1) * P],
                        in_=h_grid[:, t * D + k * P: t * D + (k + 1) * P],
                        identity=ident[:])
                hT = workp.tile([P, D], bf16, name="hT", tag="hT")
                nc.vector.tensor_copy(hT[:], pt[:])
                pm = psump.tile([P, D], f32, name="pm", tag="pm")
                for k in range(DC):
                    nc.tensor.matmul(
                        pm[:],
                        lhsT=hT[:, k * P:(k + 1) * P],
                        rhs=wt_sb[:, (l * DC + k) * D:(l * DC + k + 1) * D],
                        start=(k == 0), stop=(k == DC - 1))
                nc.scalar.copy(agg_grid[:, tsl(t)], pm[:])
                jj = junkp.tile([P, D], f32, name="junk", tag="junk")
                nc.scalar.activation(jj[:], pm[:], AF.Square,
                                     accum_out=mxn2_g[l][:, t:t + 1])

            # ---- exchange tensors: two halves per layer ----
            ts_loc = [[dramp.tile([HR, D], EX, name=f"ts_loc{l}_{h}",
                                  tag=f"ts_loc{l}_{h}") for h in range(2)]
                      for l in range(2)]
            ts_full = [[dramp.tile([NCORES * HR, D], EX, addr_space="Shared",
                                   name=f"ts_full{l}_{h}",
                                   tag=f"ts_full{l}_{h}") for h in range(2)]
                       for l in range(2)]
            y2col = gridp.tile([P, 1], f32, name="y2col", tag="y2col")
            avt = {}

            def GA(tag):
                if tag not in avt:
                    avt[tag] = G(tag)
                return avt[tag]

            def artanh2h(nm, xx, cs):
                xcl = GA(nm + "_xcl")
                nc.vector.tensor_scalar_min(xcl[:, cs], xx[:, cs],
                                            1.0 - ATEPS)
                a1 = GA(nm + "_a1")
                nc.scalar.activation(a1[:, cs], xcl[:, cs], AF.Ln,
                                     bias=1.0, scale=1.0)
                omx = GA(nm + "_omx")
                nc.vector.tensor_scalar(out=omx[:, cs], in0=xcl[:, cs],
                                        scalar1=-1.0, scalar2=1.0,
                                        op0=OP.mult, op1=OP.add)
                a2 = GA(nm + "_a2")
                nc.scalar.activation(a2[:, cs], omx[:, cs], AF.Ln)
                at2 = GA(nm + "_at2")
                nc.vector.tensor_tensor(out=at2[:, cs], in0=a1[:, cs],
                                        in1=a2[:, cs], op=OP.subtract)
                return at2

            def emit_stageA(l, hh):
                """mobius_add scalar stages + passes 2/3 + ts out for half hh
                of layer l."""
                cs = slice(hh * TH, (hh + 1) * TH)
                trng = range(hh * TH, (hh + 1) * TH)
                first = hh == 0
                y_ap = y_sb[:, l * D:(l + 1) * D]
                mxn2 = mxn2_g[l]
                if first:
                    jy = junkp.tile([P, D], f32, name="junk", tag="junk")
                    nc.scalar.activation(jy[:], y_ap, AF.Square,
                                         accum_out=y2col[:])
                # stage 1
                xn = GA("xn")
                nc.scalar.activation(xn[:, cs], hn2[:, cs], AF.Sqrt)
                mxn = GA("mxn")
                nc.scalar.activation(mxn[:, cs], mxn2[:, cs], AF.Sqrt)
                xng = GA("xng")
                nc.vector.tensor_scalar_max(xng[:, cs], xn[:, cs], EPS)
                xrec = GA("xrec")
                nc.vector.reciprocal(xrec[:, cs], xng[:, cs])
                at2 = artanh2h("s1", xn, cs)
                rr2 = GA("rr2")
                nc.vector.tensor_tensor(out=rr2[:, cs], in0=at2[:, cs],
                                        in1=xrec[:, cs], op=OP.mult)
                mxng = GA("mxng")
                nc.vector.tensor_scalar_max(mxng[:, cs], mxn[:, cs], EPS)
                mrec = GA("mrec")
                nc.vector.reciprocal(mrec[:, cs], mxng[:, cs])
                cc = GA("cc")
                nc.vector.scalar_tensor_tensor(out=cc[:, cs],
                                               in0=mxn[:, cs], scalar=0.5,
                                               in1=rr2[:, cs],
                                               op0=OP.mult, op1=OP.mult)
                tch = GA("tch")
                nc.scalar.activation(tch[:, cs], cc[:, cs], AF.Tanh)
                tcg = GA("tcg")
                nc.vector.tensor_scalar_max(tcg[:, cs], tch[:, cs], EPS)
                tcrec = GA("tcrec")
                nc.vector.reciprocal(tcrec[:, cs], tcg[:, cs])
                psA = GA("psA")
                nc.vector.tensor_scalar(out=psA[:, cs], in0=tcrec[:, cs],
                                        scalar1=MN, scalar2=1.0,
                                        op0=OP.mult, op1=OP.min)
                sp0 = GA("sp0")
                nc.vector.tensor_tensor(out=sp0[:, cs], in0=tch[:, cs],
                                        in1=mrec[:, cs], op=OP.mult)
                spg = GA("spg")
                nc.vector.tensor_tensor(out=spg[:, cs], in0=sp0[:, cs],
                                        in1=psA[:, cs], op=OP.mult)
                tcm = GA("tcm")
                nc.vector.tensor_scalar_min(tcm[:, cs], tch[:, cs], MN)
                x2 = GA("x2")
                nc.vector.tensor_tensor(out=x2[:, cs], in0=tcm[:, cs],
                                        in1=tcm[:, cs], op=OP.mult)
                # pass 2: xy = sum((sp*mx) . y)
                xy = GA("xy")
                for t in trng:
                    jx = junkp.tile([P, D], f32, name="junk", tag="junk")
                    nc.vector.scalar_tensor_tensor(
                        out=jx[:], in0=agg_grid[:, tsl(t)],
                        scalar=spg[:, t:t + 1], in1=y_ap,
                        op0=OP.mult, op1=OP.mult,
                        accum_out=xy[:, t:t + 1])
                # stage 2
                t0 = GA("t0")
                nc.vector.tensor_scalar(out=t0[:, cs], in0=xy[:, cs],
                                        scalar1=2.0, scalar2=1.0,
                                        op0=OP.mult, op1=OP.add)
                ag = GA("ag")
                nc.vector.tensor_scalar_add(ag[:, cs], t0[:, cs],
                                            y2col[:, 0:1])
                d0 = GA("d0")
                nc.vector.tensor_scalar_mul(d0[:, cs], x2[:, cs],
                                            y2col[:, 0:1])
                d1 = GA("d1")
                nc.vector.tensor_tensor(out=d1[:, cs], in0=d0[:, cs],
                                        in1=t0[:, cs], op=OP.add)
                dg = GA("dg")
                nc.vector.tensor_scalar_max(dg[:, cs], d1[:, cs], EPS)
                dinv = GA("dinv")
                nc.vector.reciprocal(dinv[:, cs], dg[:, cs])
                alpha = GA("alpha")
                nc.vector.tensor_tensor(out=alpha[:, cs], in0=ag[:, cs],
                                        in1=dinv[:, cs], op=OP.mult)
                bsc = GA("bsc")
                nc.vector.tensor_scalar(out=bsc[:, cs], in0=x2[:, cs],
                                        scalar1=-1.0, scalar2=1.0,
                                        op0=OP.mult, op1=OP.add)
                beta = GA("beta")
                nc.vector.tensor_tensor(out=beta[:, cs], in0=bsc[:, cs],
                                        in1=dinv[:, cs], op=OP.mult)
                alphasp = GA("alphasp")
                nc.vector.tensor_tensor(out=alphasp[:, cs],
                                        in0=alpha[:, cs], in1=spg[:, cs],
                                        op=OP.mult)
                # pass 3: u = alphasp*mx + beta*y (into h_grid)
                un2 = GA("un2")
                for t in trng:
                    t1 = workp.tile([P, D], f32, name="t1", tag="t1")
                    if l == 0:
                        nc.gpsimd.tensor_scalar_mul(t1[:], y_ap,
                                                    beta[:, t:t + 1])
                    else:
                        nc.vector.tensor_scalar_mul(t1[:], y_ap,
                                                    beta[:, t:t + 1])
                    us = h_grid[:, tsl(t)]
                    nc.vector.scalar_tensor_tensor(
                        out=us, in0=agg_grid[:, tsl(t)],
                        scalar=alphasp[:, t:t + 1], in1=t1[:],
                        op0=OP.mult, op1=OP.add)
                    ju = junkp.tile([P, D], f32, name="junk", tag="junk")
                    nc.scalar.activation(ju[:], us, AF.Square,
                                         accum_out=un2[:, t:t + 1])
                # stage 3: gamma
                un = GA("un")
                nc.scalar.activation(un[:, cs], un2[:, cs], AF.Sqrt)
                ung = GA("ung")
                nc.vector.tensor_scalar_max(ung[:, cs], un[:, cs], EPS)
                urec = GA("urec")
                nc.vector.reciprocal(urec[:, cs], ung[:, cs])
                h2n = GA("h2n")
                nc.vector.tensor_scalar_min(h2n[:, cs], un[:, cs], MN)
                at2u = artanh2h("s3", h2n, cs)
                h2ng = GA("h2ng")
                nc.vector.tensor_scalar_max(h2ng[:, cs], h2n[:, cs], EPS)
                hrec = GA("hrec")
                nc.vector.reciprocal(hrec[:, cs], h2ng[:, cs])
                lam2 = GA("lam2")
                nc.vector.tensor_tensor(out=lam2[:, cs], in0=at2u[:, cs],
                                        in1=hrec[:, cs], op=OP.mult)
                pst = GA("pst")
                nc.vector.tensor_scalar(out=pst[:, cs], in0=urec[:, cs],
                                        scalar1=MN, scalar2=1.0,
                                        op0=OP.mult, op1=OP.min)
                gm0 = GA("gm0")
                nc.vector.scalar_tensor_tensor(out=gm0[:, cs],
                                               in0=lam2[:, cs], scalar=0.5,
                                               in1=pst[:, cs],
                                               op0=OP.mult, op1=OP.mult)
                gam = GA("gam")
                nc.vector.tensor_tensor(out=gam[:, cs], in0=gm0[:, cs],
                                        in1=disg[:, cs], op=OP.mult)
                # ts tiles out (ScalarE: copy with per-partition scale)
                for t in trng:
                    tst = workp.tile([P, D], EX, name="tst", tag="tst")
                    nc.scalar.activation(tst[:], h_grid[:, tsl(t)],
                                         AF.Copy, scale=gam[:, t:t + 1])
                    t_rel = t - hh * TH
                    nc.sync.dma_start(
                        out=ts_loc[l][hh][t_rel * P:(t_rel + 1) * P, :],
                        in_=tst[:])

            def emit_AG(l, hh):
                nc.gpsimd.collective_compute(
                    "AllGather", OP.bypass,
                    replica_groups=[list(range(NCORES))],
                    ins=[ts_loc[l][hh].opt()], outs=[ts_full[l][hh].opt()])

            bvt = {}

            def GB(tag):
                if tag not in bvt:
                    bvt[tag] = G(tag)
                return bvt[tag]

            def expmap_grid_cs(nm, n2, cs, with_dis=True):
                """sig2 columns cs of expmap0(dis*agg) incl. dst-side dis
                (or plain expmap0 scaling when with_dis=False);
                also writes hn2[:, cs]."""
                n = GB(nm + "_n")
                nc.scalar.activation(n[:, cs], n2[:, cs], AF.Sqrt)
                if with_dis:
                    npr = GB(nm + "_npr")
                    nc.vector.tensor_tensor(out=npr[:, cs], in0=n[:, cs],
                                            in1=disg[:, cs], op=OP.mult)
                else:
                    npr = n
                ng = GB(nm + "_ng")
                nc.vector.tensor_scalar_max(ng[:, cs], npr[:, cs], EPS)
                tn = GB(nm + "_tn")
                nc.scalar.activation(tn[:, cs], npr[:, cs], AF.Tanh)
                rec = GB(nm + "_rec")
                nc.vector.reciprocal(rec[:, cs], ng[:, cs])
                sc0 = GB(nm + "_sc0")
                nc.vector.tensor_tensor(out=sc0[:, cs], in0=tn[:, cs],
                                        in1=rec[:, cs], op=OP.mult)
                tng = GB(nm + "_tng")
                nc.vector.tensor_scalar_max(tng[:, cs], tn[:, cs], EPS)
                trec = GB(nm + "_trec")
                nc.vector.reciprocal(trec[:, cs], tng[:, cs])
                ps = GB(nm + "_ps")
                nc.vector.tensor_scalar(out=ps[:, cs], in0=trec[:, cs],
                                        scalar1=MN, scalar2=1.0,
                                        op0=OP.mult, op1=OP.min)
                sig = GB(nm + "_sig")
                nc.vector.tensor_tensor(out=sig[:, cs], in0=sc0[:, cs],
                                        in1=ps[:, cs], op=OP.mult)
                if with_dis:
                    sig2 = GB(nm + "_sig2")
                    nc.vector.tensor_tensor(out=sig2[:, cs], in0=sig[:, cs],
                                            in1=disg[:, cs], op=OP.mult)
                    sig = sig2
                tnm = GB(nm + "_tnm")
                nc.vector.tensor_scalar_min(tnm[:, cs], tn[:, cs], MN)
                nc.vector.tensor_tensor(out=hn2[:, cs], in0=tnm[:, cs],
                                        in1=tnm[:, cs], op=OP.mult)
                return sig

            # ---- init: h = expmap0(x), one half at a time ----
            n2i = G("n2i")

            def emit_init(hh):
                cs = slice(hh * TH, (hh + 1) * TH)
                for t in range(hh * TH, (hh + 1) * TH):
                    t_rel = t - hh * TH
                    xs = xstage[:, t_rel * D:(t_rel + 1) * D]
                    nc.sync.dma_start(out=xs, in_=x_in[t * P:(t + 1) * P, :])
                    jj = junkp.tile([P, D], f32, name="junk", tag="junk")
                    nc.scalar.activation(jj[:], xs, AF.Square,
                                         accum_out=n2i[:, t:t + 1])
                sig0 = expmap_grid_cs("em0", n2i, cs, with_dis=False)
                for t in range(hh * TH, (hh + 1) * TH):
                    t_rel = t - hh * TH
                    xs = xstage[:, t_rel * D:(t_rel + 1) * D]
                    nc.vector.tensor_scalar_mul(h_grid[:, tsl(t)], xs,
                                                sig0[:, t:t + 1])

            # ---- phaseB ----
            # chunk -> gather bookkeeping (per class stream)
            off0 = [0]
            off1 = [0]
            for b in range(T):
                off0.append(off0[-1] + c0[b])
                off1.append(off1[-1] + c1[b])
            colof = [0]
            for b in range(T):
                colof.append(colof[-1] + c0[b] + c1[b])
            gsb = [g0_sb, g1_sb]
            qctr = [0]

            def emit_phaseB(l):
                an2 = G("an2")
                cur = {0: (-1, None), 1: (-1, None)}  # stream -> (g, tile)
                for b in range(T):
                    nch = c0[b] + c1[b]
                    S = sblkp.tile([P, CMAX * P], EX, name="S", tag="S")
                    nc.vector.tensor_tensor(
                        out=S[:, 0:nch * P].rearrange(
                            "p (c j) -> p c j", c=nch),
                        in0=edst_sb[:, colof[b]:colof[b] + nch].to_broadcast(
                            [P, nch, P]),
                        in1=iota_sb[:].rearrange("p (o j) -> p o j", o=1)
                            .to_broadcast([P, nch, P]),
                        op=OP.is_equal)
                    pa = psagp.tile([P, D], f32, name="pa", tag="pa")
                    k = 0
                    for cls in range(2):
                        nck = c0[b] if cls == 0 else c1[b]
                        ofs = off0 if cls == 0 else off1
                        for c in range(nck):
                            j = ofs[b] + c
                            g, s = divmod(j, CPG)
                            if cur[cls][0] != g:
                                m = msgp.tile([P, CPG * D], EX, name="m",
                                              tag="m")
                                nc.gpsimd.dma_gather(
                                    m[:].rearrange("p (c e) -> p c e",
                                                   c=CPG),
                                    ts_full[l][cls],
                                    gsb[cls][:, g * (GN // 16):
                                             (g + 1) * (GN // 16)],
                                    GN, GN, D, queue_num=qctr[0] % NQ)
                                qctr[0] += 1
                                cur[cls] = (g, m)
                            m = cur[cls][1]
                            nc.tensor.matmul(
                                pa[:],
                                lhsT=S[:, k * P:(k + 1) * P],
                                rhs=m[:, s * D:(s + 1) * D],
                                start=(k == 0), stop=(k == nch - 1))
                            k += 1
                    jj = junkp.tile([P, D], f32, name="junk", tag="junk")
                    nc.scalar.activation(jj[:], pa[:], AF.Square,
                                         accum_out=an2[:, b:b + 1])
                    # defer expmap scaling to the group epilogue
                    nc.scalar.copy(h_grid[:, tsl(b)], pa[:])
                    if (b + 1) % GRP == 0:
                        g0 = b + 1 - GRP
                        cs = slice(g0, b + 1)
                        sig = expmap_grid_cs("emB", an2, cs)
                        for t in range(g0, b + 1):
                            nc.vector.tensor_scalar_mul(
                                h_grid[:, tsl(t)], h_grid[:, tsl(t)],
                                sig[:, t:t + 1])
                        if l == 0:
                            for t in range(g0, b + 1):
                                emit_pass1(1, t)
                        else:
                            for t in range(g0, b + 1):
                                nc.sync.dma_start(
                                    out=out_ext[t * P:(t + 1) * P, :],
                                    in_=h_grid[:, tsl(t)])
                    # inject layer-1 stageA half 0 + its AG mid-phaseB(0):
                    # first group boundary at which pass1(1, 0..TH-1) exists
                    if (l == 0 and (b + 1) % GRP == 0
                            and b + 1 >= TH and b + 1 - GRP < TH):
                        emit_stageA(1, 0)
                        emit_AG(1, 0)

            # ================= emission =================
            # layer 0 phase A: per half, then its AllGather
            for hh in range(2):
                emit_init(hh)
                for t in range(hh * TH, (hh + 1) * TH):
                    emit_pass1(0, t)
                emit_stageA(0, hh)
                emit_AG(0, hh)
            # layer 0 phase B (embeds layer-1 pass1 + stageA half0 + AG)
            emit_phaseB(0)
            # layer 1 phase A second half + exchange
            emit_stageA(1, 1)
            emit_AG(1, 1)
            # layer 1 phase B
            emit_phaseB(1)

    nc.compile()
    return nc


def _get_program(T, DC, c0, c1, NG0, NG1):
    key = (T, DC, tuple(c0), tuple(c1), NG0, NG1)
    if key not in _prog_cache:
        _prog_cache[key] = _build_program(T, DC, c0, c1, NG0, NG1)
    return _prog_cache[key]


# ----------------------------------------------------------------- entry

def run(inputs, trace=False, trace_kwargs=None):
    x = np.asarray(inputs["x"], np.float32)
    ei = np.asarray(inputs["edge_index"])
    W1 = np.asarray(inputs["W1"], np.float32)
    b1 = np.asarray(inputs["b1"], np.float32)
    W2 = np.asarray(inputs["W2"], np.float32)
    b2 = np.asarray(inputs["b2"], np.float32)
    N, D = x.shape
    assert D % P == 0
    meta, per_core = _host_prep(x, ei)
    T, DC = meta["T"], D // P
    c0, c1, NG0, NG1 = meta["c0"], meta["c1"], meta["NG0"], meta["NG1"]
    n_loc, perm = meta["n_loc"], meta["perm"]

    wt = np.stack([np.ascontiguousarray(W1.T), np.ascontiguousarray(W2.T)])
    wt = wt.astype(ml_dtypes.bfloat16)
    y = np.stack([np.tile(_np_expmap0(b1)[None, :], (P, 1)),
                  np.tile(_np_expmap0(b2)[None, :], (P, 1))])

    nc = _get_program(T, DC, c0, c1, NG0, NG1)
    in_maps = []
    for r in range(NCORES):
        m = dict(per_core[r])
        m["wt"] = wt
        m["y"] = y
        in_maps.append(m)

    kwargs = {}
    if trace:
        kwargs = dict(trace=True, trace_kwargs=trace_kwargs or {})
    res = run_bass_kernel_spmd(nc, in_maps, list(range(NCORES)), **kwargs)
    out = np.empty((N, D), np.float32)
    for r in range(NCORES):
        res_r = np.asarray(res.results[r]["out"])
        pr = perm[r]
        valid = pr >= 0
        out[r * n_loc + pr[valid]] = res_r[np.nonzero(valid)[0]]
    return out, res


def kernel(**inputs):
    out, _ = run(inputs)
    return out


# revision 11
# speedup vs baseline: 1.1039x; 1.1039x over previous
"""Trainium2 Bass kernel for the 2-layer hyperbolic (Poincare ball) GCN encoder.

Strategy (8 NeuronCores, SPMD):
  - Nodes sharded across cores (2500 rows/core, padded to 2560 = 20 tiles of 128),
    with a per-core degree-balanced permutation so every 128-destination block
    has ~equal edge count.
  - Weights replicated (bf16); dense mobius_matvec/mobius_add/logmap0 computed on
    the owned shard with all per-row reductions fused into per-partition scalar
    "grid" tensors of shape [128, T].
  - Per-layer exchange: tangent features (pre-scaled by deg^-0.5 on the source
    side) are AllGathered in bf16 across the 8 cores, SPLIT INTO TWO HALF-SHARD
    COLLECTIVES so each half is triggered as soon as its stageA tiles finish.
    A tiny warmup AllGather at kernel start absorbs the one-time CC-library
    load / mesh setup (~150us on the profiled baseline).
  - Edges partitioned by destination and CLASSIFIED BY SOURCE HALF: within each
    128-destination block, chunks whose sources live in half 0 are processed
    first, so phaseB's gathers begin while the half-1 AllGather is in flight.
    Per-block per-class chunk counts are ragged (baked at build time).
  - Messages fetched with dma_gather (1024 rows per instruction) round-robined
    over 4 SWDGE queues with rotating msg buffers; segment-sum on TensorE via
    0/1 selection matrices accumulated in PSUM.
  - Layer-1 phaseA is emitted inside layer-0 phaseB (pass1 in the group
    epilogues; stageA half 0 + its AllGather right after block T/2-1), so the
    layer-1 exchange is fully hidden behind layer-0's gather/matmul pipeline.
"""
import os
import numpy as np
import ml_dtypes

import concourse.bass as bass
import concourse.bacc as bacc
import concourse.tile as tile
import concourse.mybir as mybir
from concourse.bass_utils import run_bass_kernel_spmd
from concourse.masks import make_identity

NCORES = 8
P = 128
GN = 1024            # indices per dma_gather
CPG = GN // P        # chunks per gather
NQ = 4               # SWDGE queues
MN = 1.0 - 4e-3
EPS = 1e-15
ATEPS = 1e-7

f32 = mybir.dt.float32
bf16 = mybir.dt.bfloat16
i16 = mybir.dt.int16
AF = mybir.ActivationFunctionType
OP = mybir.AluOpType

_prog_cache = {}


# ----------------------------------------------------------------- host side

def _np_expmap0(u):
    u = np.asarray(u, np.float32)
    n = max(float(np.linalg.norm(u)), EPS)
    v = (np.tanh(n) * u / n).astype(np.float32)
    nn = max(float(np.linalg.norm(v)), EPS)
    if nn > MN:
        v = (v / nn * MN).astype(np.float32)
    return v


def _wrap_idx(lin, NGs):
    """[NGs*GN] linear slot order -> int16 [128, NGs*(GN//16)] wrapped."""
    w = lin.reshape(NGs, GN // 16, 16).transpose(2, 0, 1).reshape(16, -1)
    return np.tile(w.astype(np.int16), (8, 1))


def _host_prep(x, edge_index):
    x = np.asarray(x, np.float32)
    ei = np.asarray(edge_index)
    N, D = x.shape
    assert N % NCORES == 0
    n_loc = N // NCORES
    T = (n_loc + P - 1) // P
    assert T % 2 == 0
    TH = T // 2
    HR = TH * P                      # rows per half per core
    n_pad = T * P
    assert NCORES * HR <= 32767, "indices must fit int16"

    loops = np.arange(N, dtype=ei.dtype)
    ei = np.concatenate([ei, np.stack([loops, loops])], axis=1)
    row, col = ei[0].astype(np.int64), ei[1].astype(np.int64)
    deg = np.bincount(col, minlength=N).astype(np.float32)
    dis = (deg ** -0.5).astype(np.float32)

    # --- per-core degree-balanced slot permutation -------------------------
    inv_perm = np.empty((NCORES, n_loc), np.int64)   # local node -> slot
    perm = np.full((NCORES, n_pad), -1, np.int64)    # slot -> local node
    for r in range(NCORES):
        dloc = deg[r * n_loc:(r + 1) * n_loc]
        order = np.argsort(-dloc, kind="stable")
        ids = np.full(n_pad, -1, np.int64)
        ids[:n_loc] = order
        ids = ids.reshape(P, T)
        ids[1::2] = ids[1::2, ::-1]
        for b in range(T):
            blk_nodes = ids[:, b]
            for j, nd in enumerate(blk_nodes):
                slot = b * P + j
                perm[r, slot] = nd
                if nd >= 0:
                    inv_perm[r, nd] = slot

    src_core = row // n_loc
    src_slot = inv_perm[src_core, row % n_loc]
    src_half = src_slot // HR                         # 0 or 1
    src_row = src_core * HR + (src_slot % HR)         # row in ts_full[half]
    dst_core = col // n_loc
    dst_slot = inv_perm[dst_core, col % n_loc]

    blk = dst_slot // P + dst_core * T
    order = np.lexsort((src_row, src_half, blk))
    src_s = src_row[order]
    half_s = src_half[order]
    blk_s = blk[order]
    dstrel_s = (dst_slot[order] % P).astype(np.float32)

    blk_counts = np.bincount(blk_s, minlength=NCORES * T)
    bounds = np.concatenate([[0], np.cumsum(blk_counts)])

    # Ragged per-block per-class chunk layout. Identical across cores in
    # CHUNK COUNTS is required (one SPMD program) -> use the max per (b,cls)
    # over cores and pad. Slot streams are per-core.
    c_cnt = np.zeros((NCORES, T, 2), np.int64)
    for r in range(NCORES):
        for b in range(T):
            lo, hi = bounds[r * T + b], bounds[r * T + b + 1]
            h = half_s[lo:hi]
            n0 = int((h == 0).sum())
            n1 = int(hi - lo - n0)
            c_cnt[r, b, 0] = (n0 + P - 1) // P
            c_cnt[r, b, 1] = (n1 + P - 1) // P
    cmax = c_cnt.max(axis=0)                 # [T, 2] chunks per block/class
    c0 = [int(v) for v in cmax[:, 0]]
    c1 = [int(v) for v in cmax[:, 1]]
    tot0, tot1 = sum(c0), sum(c1)
    NG0 = (tot0 * P + GN - 1) // GN
    NG1 = (tot1 * P + GN - 1) // GN

    # per-core slot streams (gather indices + dst-rel for S build)
    gl0 = np.zeros((NCORES, NG0 * GN), np.int64)
    gl1 = np.zeros((NCORES, NG1 * GN), np.int64)
    edst = np.full((NCORES, P, tot0 + tot1), -1.0, np.float32)
    off0 = np.concatenate([[0], np.cumsum(c0)])
    off1 = np.concatenate([[0], np.cumsum(c1)])
    colof = np.concatenate([[0], np.cumsum([a + b for a, b in zip(c0, c1)])])
    for r in range(NCORES):
        for b in range(T):
            lo, hi = bounds[r * T + b], bounds[r * T + b + 1]
            h = half_s[lo:hi]
            for cls, (glin, offs) in enumerate(((gl0, off0), (gl1, off1))):
                sel = np.nonzero(h == cls)[0] + lo
                L = len(sel)
                nch = (c0[b] if cls == 0 else c1[b])
                pad = np.zeros(nch * P, np.int64)
                pad[:L] = src_s[sel]
                glin[r, offs[b] * P:(offs[b] + nch) * P] = pad
                dpad = np.full(nch * P, -1.0, np.float32)
                dpad[:L] = dstrel_s[sel]
                cbase = colof[b] + (0 if cls == 0 else c0[b])
                edst[r][:, cbase:cbase + nch] = dpad.reshape(nch, P).T

    idx0 = np.stack([_wrap_idx(gl0[r], NG0) for r in range(NCORES)])
    idx1 = np.stack([_wrap_idx(gl1[r], NG1) for r in range(NCORES)])

    dis_loc = np.zeros((NCORES, P, T), np.float32)
    x_loc = np.zeros((NCORES, n_pad, D), np.float32)
    for r in range(NCORES):
        for slot in range(n_pad):
            nd = perm[r, slot]
            if nd >= 0:
                dis_loc[r, slot % P, slot // P] = dis[r * n_loc + nd]
                x_loc[r, slot] = x[r * n_loc + nd]

    iota = np.tile(np.arange(P, dtype=np.float32)[None, :], (P, 1))
    meta = dict(N=N, D=D, n_loc=n_loc, T=T, n_pad=n_pad, perm=perm,
                c0=c0, c1=c1, NG0=NG0, NG1=NG1)
    per_core = [dict(x=x_loc[r].astype(ml_dtypes.bfloat16),
                     dis=dis_loc[r],
                     gidx0=idx0[r], gidx1=idx1[r],
                     edst=edst[r].astype(ml_dtypes.bfloat16),
                     iota=iota.astype(ml_dtypes.bfloat16))
                for r in range(NCORES)]
    return meta, per_core


# --------------------------------------------------------------- device side

def _build_program(T, DC, c0, c1, NG0, NG1):
    D = DC * P
    TH = T // 2
    HR = TH * P
    NPAD = T * P
    EX = bf16
    TOT = sum(c0) + sum(c1)
    CMAX = max(a + b for a, b in zip(c0, c1))
    GRP = int(os.environ.get("KGRP", "5"))
    MSGB = int(os.environ.get("KMSGB", "8"))
    PRE = int(os.environ.get("KPRE", "4"))

    nc = bacc.Bacc("TRN2", target_bir_lowering=False, debug=False,
                   num_devices=NCORES, num_swdge_queues=NQ,
                   dynamic_dma_scratch_size=int(os.environ.get("KSCRATCH",
                                                               "16384")))

    x_in = nc.dram_tensor("x", [NPAD, D], bf16, kind="ExternalInput")
    wt_in = nc.dram_tensor("wt", [2, D, D], bf16, kind="ExternalInput")
    y_in = nc.dram_tensor("y", [2, P, D], f32, kind="ExternalInput")
    iota_in = nc.dram_tensor("iota", [P, P], bf16, kind="ExternalInput")
    dis_in = nc.dram_tensor("dis", [P, T], f32, kind="ExternalInput")
    g0_in = nc.dram_tensor("gidx0", [P, NG0 * (GN // 16)], i16,
                           kind="ExternalInput")
    g1_in = nc.dram_tensor("gidx1", [P, NG1 * (GN // 16)], i16,
                           kind="ExternalInput")
    edst_in = nc.dram_tensor("edst", [P, TOT], bf16, kind="ExternalInput")
    out_ext = nc.dram_tensor("out", [NPAD, D], f32, kind="ExternalOutput")

    with tile.TileContext(nc) as tc:
        with (
            tc.tile_pool(name="const", bufs=1) as constp,
            tc.tile_pool(name="grid", bufs=1) as gridp,
            tc.tile_pool(name="big", bufs=1) as bigp,
            tc.tile_pool(name="work", bufs=3) as workp,
            tc.tile_pool(name="junk", bufs=3) as junkp,
            tc.tile_pool(name="msgs", bufs=MSGB) as msgp,
            tc.tile_pool(name="sblk", bufs=2) as sblkp,
            tc.tile_pool(name="psum", bufs=2, space="PSUM") as psump,
            tc.tile_pool(name="psag", bufs=3, space="PSUM") as psagp,
            tc.tile_pool(name="dram", bufs=1, space="DRAM") as dramp,
        ):
            # ---- CC warmup: tiny AllGather issued before anything else ----
            warm_l = dramp.tile([P, 16], bf16, name="warm_l", tag="warm_l")
            warm_g = dramp.tile([NCORES * P, 16], bf16, addr_space="Shared",
                                name="warm_g", tag="warm_g")
            nc.sync.dma_start(out=warm_l[:], in_=iota_in[:, 0:16])
            nc.gpsimd.collective_compute(
                "AllGather", OP.bypass,
                replica_groups=[list(range(NCORES))],
                ins=[warm_l.opt()], outs=[warm_g.opt()])

            # ---- constants ----
            wt_sb = constp.tile([P, 2 * DC * D], bf16, name="wt", tag="wt")
            for l in range(2):
                for k in range(DC):
                    nc.sync.dma_start(
                        out=wt_sb[:, (l * DC + k) * D:(l * DC + k + 1) * D],
                        in_=wt_in[l, k * P:(k + 1) * P, :])
            y_sb = constp.tile([P, 2 * D], f32, name="y", tag="y")
            nc.sync.dma_start(out=y_sb[:, 0:D], in_=y_in[0])
            nc.sync.dma_start(out=y_sb[:, D:2 * D], in_=y_in[1])
            iota_sb = constp.tile([P, P], bf16, name="iota", tag="iota")
            nc.sync.dma_start(out=iota_sb[:], in_=iota_in[:, :])
            ident = constp.tile([P, P], f32, name="ident", tag="ident")
            make_identity(nc, ident[:])
            disg = constp.tile([P, T], f32, name="dis", tag="dis")
            nc.sync.dma_start(out=disg[:], in_=dis_in[:, :])
            g0_sb = constp.tile([P, NG0 * (GN // 16)], i16, name="g0",
                                tag="g0")
            nc.sync.dma_start(out=g0_sb[:], in_=g0_in[:, :])
            g1_sb = constp.tile([P, NG1 * (GN // 16)], i16, name="g1",
                                tag="g1")
            nc.sync.dma_start(out=g1_sb[:], in_=g1_in[:, :])
            edst_sb = constp.tile([P, TOT], bf16, name="edst", tag="edst")
            nc.sync.dma_start(out=edst_sb[:], in_=edst_in[:, :])

            # ---- persistent big tensors ----
            h_grid = bigp.tile([P, T * D], f32, name="h", tag="h")  # h then u
            agg_grid = bigp.tile([P, T * D], bf16, name="agg", tag="agg")
            xstage = bigp.tile([P, TH * D], bf16, name="xs", tag="xs")
            hn2 = gridp.tile([P, T], f32, name="hn2", tag="hn2")

            def G(tag):
                return gridp.tile([P, T], f32, name=tag, tag=tag)

            def tsl(t):
                return slice(t * D, (t + 1) * D)

            mxn2_g = [G("mxn2_0"), G("mxn2_1")]

            def emit_pass1(l, t):
                pt = psump.tile([P, D], f32, name="pt", tag="pt")
                for k in range(DC):
                    nc.tensor.transpose(
                        out=pt[:, k * P:(k + 1) * P],
                        in_=h_grid[:, t * D + k * P: t * D + (k + 1) * P],
                        identity=ident[:])
                hT = workp.tile([P, D], bf16, name="hT", tag="hT")
                nc.vector.tensor_copy(hT[:], pt[:])
                pm = psump.tile([P, D], f32, name="pm", tag="pm")
                for k in range(DC):
                    nc.tensor.matmul(
                        pm[:],
                        lhsT=hT[:, k * P:(k + 1) * P],
                        rhs=wt_sb[:, (l * DC + k) * D:(l * DC + k + 1) * D],
                        start=(k == 0), stop=(k == DC - 1))
                nc.scalar.copy(agg_grid[:, tsl(t)], pm[:])
                jj = junkp.tile([P, D], f32, name="junk", tag="junk")
                nc.scalar.activation(jj[:], pm[:], AF.Square,
                                     accum_out=mxn2_g[l][:, t:t + 1])

            # ---- exchange tensors: two halves per layer ----
            ts_loc = [[dramp.tile([HR, D], EX, name=f"ts_loc{l}_{h}",
                                  tag=f"ts_loc{l}_{h}") for h in range(2)]
                      for l in range(2)]
            ts_full = [[dramp.tile([NCORES * HR, D], EX, addr_space="Shared",
                                   name=f"ts_full{l}_{h}",
                                   tag=f"ts_full{l}_{h}") for h in range(2)]
                       for l in range(2)]
            y2col = gridp.tile([P, 1], f32, name="y2col", tag="y2col")
            avt = {}

            def GA(tag):
                if tag not in avt:
                    avt[tag] = G(tag)
                return avt[tag]

            def artanh2h(nm, xx, cs):
                xcl = GA(nm + "_xcl")
                nc.vector.tensor_scalar_min(xcl[:, cs], xx[:, cs],
                                            1.0 - ATEPS)
                a1 = GA(nm + "_a1")
                nc.scalar.activation(a1[:, cs], xcl[:, cs], AF.Ln,
                                     bias=1.0, scale=1.0)
                omx = GA(nm + "_omx")
                nc.vector.tensor_scalar(out=omx[:, cs], in0=xcl[:, cs],
                                        scalar1=-1.0, scalar2=1.0,
                                        op0=OP.mult, op1=OP.add)
                a2 = GA(nm + "_a2")
                nc.scalar.activation(a2[:, cs], omx[:, cs], AF.Ln)
                at2 = GA(nm + "_at2")
                nc.vector.tensor_tensor(out=at2[:, cs], in0=a1[:, cs],
                                        in1=a2[:, cs], op=OP.subtract)
                return at2

            def emit_stageA(l, hh):
                """mobius_add scalar stages + passes 2/3 + ts out for half hh
                of layer l."""
                cs = slice(hh * TH, (hh + 1) * TH)
                trng = range(hh * TH, (hh + 1) * TH)
                first = hh == 0
                y_ap = y_sb[:, l * D:(l + 1) * D]
                mxn2 = mxn2_g[l]
                if first:
                    jy = junkp.tile([P, D], f32, name="junk", tag="junk")
                    nc.scalar.activation(jy[:], y_ap, AF.Square,
                                         accum_out=y2col[:])
                # stage 1
                xn = GA("xn")
                nc.scalar.activation(xn[:, cs], hn2[:, cs], AF.Sqrt)
                mxn = GA("mxn")
                nc.scalar.activation(mxn[:, cs], mxn2[:, cs], AF.Sqrt)
                xng = GA("xng")
                nc.vector.tensor_scalar_max(xng[:, cs], xn[:, cs], EPS)
                xrec = GA("xrec")
                nc.vector.reciprocal(xrec[:, cs], xng[:, cs])
                at2 = artanh2h("s1", xn, cs)
                rr2 = GA("rr2")
                nc.vector.tensor_tensor(out=rr2[:, cs], in0=at2[:, cs],
                                        in1=xrec[:, cs], op=OP.mult)
                mxng = GA("mxng")
                nc.vector.tensor_scalar_max(mxng[:, cs], mxn[:, cs], EPS)
                mrec = GA("mrec")
                nc.vector.reciprocal(mrec[:, cs], mxng[:, cs])
                cc = GA("cc")
                nc.vector.scalar_tensor_tensor(out=cc[:, cs],
                                               in0=mxn[:, cs], scalar=0.5,
                                               in1=rr2[:, cs],
                                               op0=OP.mult, op1=OP.mult)
                tch = GA("tch")
                nc.scalar.activation(tch[:, cs], cc[:, cs], AF.Tanh)
                tcg = GA("tcg")
                nc.vector.tensor_scalar_max(tcg[:, cs], tch[:, cs], EPS)
                tcrec = GA("tcrec")
                nc.vector.reciprocal(tcrec[:, cs], tcg[:, cs])
                psA = GA("psA")
                nc.vector.tensor_scalar(out=psA[:, cs], in0=tcrec[:, cs],
                                        scalar1=MN, scalar2=1.0,
                                        op0=OP.mult, op1=OP.min)
                sp0 = GA("sp0")
                nc.vector.tensor_tensor(out=sp0[:, cs], in0=tch[:, cs],
                                        in1=mrec[:, cs], op=OP.mult)
                spg = GA("spg")
                nc.vector.tensor_tensor(out=spg[:, cs], in0=sp0[:, cs],
                                        in1=psA[:, cs], op=OP.mult)
                tcm = GA("tcm")
                nc.vector.tensor_scalar_min(tcm[:, cs], tch[:, cs], MN)
                x2 = GA("x2")
                nc.vector.tensor_tensor(out=x2[:, cs], in0=tcm[:, cs],
                                        in1=tcm[:, cs], op=OP.mult)
                # pass 2: xy = sum((sp*mx) . y)
                xy = GA("xy")
                for t in trng:
                    jx = junkp.tile([P, D], f32, name="junk", tag="junk")
                    nc.vector.scalar_tensor_tensor(
                        out=jx[:], in0=agg_grid[:, tsl(t)],
                        scalar=spg[:, t:t + 1], in1=y_ap,
                        op0=OP.mult, op1=OP.mult,
                        accum_out=xy[:, t:t + 1])
                # stage 2
                t0 = GA("t0")
                nc.vector.tensor_scalar(out=t0[:, cs], in0=xy[:, cs],
                                        scalar1=2.0, scalar2=1.0,
                                        op0=OP.mult, op1=OP.add)
                ag = GA("ag")
                nc.vector.tensor_scalar_add(ag[:, cs], t0[:, cs],
                                            y2col[:, 0:1])
                d0 = GA("d0")
                nc.vector.tensor_scalar_mul(d0[:, cs], x2[:, cs],
                                            y2col[:, 0:1])
                d1 = GA("d1")
                nc.vector.tensor_tensor(out=d1[:, cs], in0=d0[:, cs],
                                        in1=t0[:, cs], op=OP.add)
                dg = GA("dg")
                nc.vector.tensor_scalar_max(dg[:, cs], d1[:, cs], EPS)
                dinv = GA("dinv")
                nc.vector.reciprocal(dinv[:, cs], dg[:, cs])
                alpha = GA("alpha")
                nc.vector.tensor_tensor(out=alpha[:, cs], in0=ag[:, cs],
                                        in1=dinv[:, cs], op=OP.mult)
                bsc = GA("bsc")
                nc.vector.tensor_scalar(out=bsc[:, cs], in0=x2[:, cs],
                                        scalar1=-1.0, scalar2=1.0,
                                        op0=OP.mult, op1=OP.add)
                beta = GA("beta")
                nc.vector.tensor_tensor(out=beta[:, cs], in0=bsc[:, cs],
                                        in1=dinv[:, cs], op=OP.mult)
                alphasp = GA("alphasp")
                nc.vector.tensor_tensor(out=alphasp[:, cs],
                                        in0=alpha[:, cs], in1=spg[:, cs],
                                        op=OP.mult)
                # pass 3: u = alphasp*mx + beta*y (into h_grid)
                un2 = GA("un2")
                for t in trng:
                    t1 = workp.tile([P, D], f32, name="t1", tag="t1")
                    nc.vector.tensor_scalar_mul(t1[:], y_ap,
                                                beta[:, t:t + 1])
                    us = h_grid[:, tsl(t)]
                    nc.vector.scalar_tensor_tensor(
                        out=us, in0=agg_grid[:, tsl(t)],
                        scalar=alphasp[:, t:t + 1], in1=t1[:],
                        op0=OP.mult, op1=OP.add)
                    ju = junkp.tile([P, D], f32, name="junk", tag="junk")
                    nc.scalar.activation(ju[:], us, AF.Square,
                                         accum_out=un2[:, t:t + 1])
                # stage 3: gamma
                un = GA("un")
                nc.scalar.activation(un[:, cs], un2[:, cs], AF.Sqrt)
                ung = GA("ung")
                nc.vector.tensor_scalar_max(ung[:, cs], un[:, cs], EPS)
                urec = GA("urec")
                nc.vector.reciprocal(urec[:, cs], ung[:, cs])
                h2n = GA("h2n")
                nc.vector.tensor_scalar_min(h2n[:, cs], un[:, cs], MN)
                at2u = artanh2h("s3", h2n, cs)
                h2ng = GA("h2ng")
                nc.vector.tensor_scalar_max(h2ng[:, cs], h2n[:, cs], EPS)
                hrec = GA("hrec")
                nc.vector.reciprocal(hrec[:, cs], h2ng[:, cs])
                lam2 = GA("lam2")
                nc.vector.tensor_tensor(out=lam2[:, cs], in0=at2u[:, cs],
                                        in1=hrec[:, cs], op=OP.mult)
                pst = GA("pst")
                nc.vector.tensor_scalar(out=pst[:, cs], in0=urec[:, cs],
                                        scalar1=MN, scalar2=1.0,
                                        op0=OP.mult, op1=OP.min)
                gm0 = GA("gm0")
                nc.vector.scalar_tensor_tensor(out=gm0[:, cs],
                                               in0=lam2[:, cs], scalar=0.5,
                                               in1=pst[:, cs],
                                               op0=OP.mult, op1=OP.mult)
                gam = GA("gam")
                nc.vector.tensor_tensor(out=gam[:, cs], in0=gm0[:, cs],
                                        in1=disg[:, cs], op=OP.mult)
                # ts tiles out (ScalarE: copy with per-partition scale)
                for t in trng:
                    tst = workp.tile([P, D], EX, name="tst", tag="tst")
                    nc.scalar.activation(tst[:], h_grid[:, tsl(t)],
                                         AF.Copy, scale=gam[:, t:t + 1])
                    t_rel = t - hh * TH
                    nc.sync.dma_start(
                        out=ts_loc[l][hh][t_rel * P:(t_rel + 1) * P, :],
                        in_=tst[:])

            def emit_AG(l, hh):
                nc.gpsimd.collective_compute(
                    "AllGather", OP.bypass,
                    replica_groups=[list(range(NCORES))],
                    ins=[ts_loc[l][hh].opt()], outs=[ts_full[l][hh].opt()])

            bvt = {}

            def GB(tag):
                if tag not in bvt:
                    bvt[tag] = G(tag)
                return bvt[tag]

            def expmap_grid_cs(nm, n2, cs, with_dis=True):
                """sig2 columns cs of expmap0(dis*agg) incl. dst-side dis
                (or plain expmap0 scaling when with_dis=False);
                also writes hn2[:, cs]."""
                n = GB(nm + "_n")
                nc.scalar.activation(n[:, cs], n2[:, cs], AF.Sqrt)
                if with_dis:
                    npr = GB(nm + "_npr")
                    nc.vector.tensor_tensor(out=npr[:, cs], in0=n[:, cs],
                                            in1=disg[:, cs], op=OP.mult)
                else:
                    npr = n
                ng = GB(nm + "_ng")
                nc.vector.tensor_scalar_max(ng[:, cs], npr[:, cs], EPS)
                tn = GB(nm + "_tn")
                nc.scalar.activation(tn[:, cs], npr[:, cs], AF.Tanh)
                rec = GB(nm + "_rec")
                nc.vector.reciprocal(rec[:, cs], ng[:, cs])
                sc0 = GB(nm + "_sc0")
                nc.vector.tensor_tensor(out=sc0[:, cs], in0=tn[:, cs],
                                        in1=rec[:, cs], op=OP.mult)
                tng = GB(nm + "_tng")
                nc.vector.tensor_scalar_max(tng[:, cs], tn[:, cs], EPS)
                trec = GB(nm + "_trec")
                nc.vector.reciprocal(trec[:, cs], tng[:, cs])
                ps = GB(nm + "_ps")
                nc.vector.tensor_scalar(out=ps[:, cs], in0=trec[:, cs],
                                        scalar1=MN, scalar2=1.0,
                                        op0=OP.mult, op1=OP.min)
                sig = GB(nm + "_sig")
                nc.vector.tensor_tensor(out=sig[:, cs], in0=sc0[:, cs],
                                        in1=ps[:, cs], op=OP.mult)
                if with_dis:
                    sig2 = GB(nm + "_sig2")
                    nc.vector.tensor_tensor(out=sig2[:, cs], in0=sig[:, cs],
                                            in1=disg[:, cs], op=OP.mult)
                    sig = sig2
                tnm = GB(nm + "_tnm")
                nc.vector.tensor_scalar_min(tnm[:, cs], tn[:, cs], MN)
                nc.vector.tensor_tensor(out=hn2[:, cs], in0=tnm[:, cs],
                                        in1=tnm[:, cs], op=OP.mult)
                return sig

            # ---- init: h = expmap0(x), one half at a time ----
            n2i = G("n2i")

            def emit_init(hh):
                cs = slice(hh * TH, (hh + 1) * TH)
                for t in range(hh * TH, (hh + 1) * TH):
                    t_rel = t - hh * TH
                    xs = xstage[:, t_rel * D:(t_rel + 1) * D]
                    nc.sync.dma_start(out=xs, in_=x_in[t * P:(t + 1) * P, :])
                    jj = junkp.tile([P, D], f32, name="junk", tag="junk")
                    nc.scalar.activation(jj[:], xs, AF.Square,
                                         accum_out=n2i[:, t:t + 1])
                sig0 = expmap_grid_cs("em0", n2i, cs, with_dis=False)
                for t in range(hh * TH, (hh + 1) * TH):
                    t_rel = t - hh * TH
                    xs = xstage[:, t_rel * D:(t_rel + 1) * D]
                    nc.vector.tensor_scalar_mul(h_grid[:, tsl(t)], xs,
                                                sig0[:, t:t + 1])

            # ---- phaseB ----
            # chunk -> gather bookkeeping (per class stream)
            off0 = [0]
            off1 = [0]
            for b in range(T):
                off0.append(off0[-1] + c0[b])
                off1.append(off1[-1] + c1[b])
            colof = [0]
            for b in range(T):
                colof.append(colof[-1] + c0[b] + c1[b])
            gsb = [g0_sb, g1_sb]
            NGs = [NG0, NG1]
            qctr = [0]
            g_issued = {}   # (l, cls) -> number of gathers issued
            g_avail = {}    # (l, cls) -> {g: msg tile}

            def issue_gather(l, cls):
                g = g_issued.get((l, cls), 0)
                if g >= NGs[cls]:
                    return
                m = msgp.tile([P, CPG * D], EX, name="m", tag="m")
                nc.gpsimd.dma_gather(
                    m[:].rearrange("p (c e) -> p c e", c=CPG),
                    ts_full[l][cls],
                    gsb[cls][:, g * (GN // 16):(g + 1) * (GN // 16)],
                    GN, GN, D, queue_num=qctr[0] % NQ)
                qctr[0] += 1
                g_issued[(l, cls)] = g + 1
                g_avail.setdefault((l, cls), {})[g] = m

            def get_mtile(l, cls, g):
                while g_issued.get((l, cls), 0) <= g:
                    issue_gather(l, cls)
                av = g_avail[(l, cls)]
                for gg in [k for k in av if k < g]:
                    del av[gg]       # consumed; release reference
                return av[g]

            def emit_phaseB(l):
                an2 = G("an2")
                for b in range(T):
                    nch = c0[b] + c1[b]
                    S = sblkp.tile([P, CMAX * P], EX, name="S", tag="S")
                    nc.vector.tensor_tensor(
                        out=S[:, 0:nch * P].rearrange(
                            "p (c j) -> p c j", c=nch),
                        in0=edst_sb[:, colof[b]:colof[b] + nch].to_broadcast(
                            [P, nch, P]),
                        in1=iota_sb[:].rearrange("p (o j) -> p o j", o=1)
                            .to_broadcast([P, nch, P]),
                        op=OP.is_equal)
                    pa = psagp.tile([P, D], f32, name="pa", tag="pa")
                    k = 0
                    for cls in range(2):
                        nck = c0[b] if cls == 0 else c1[b]
                        ofs = off0 if cls == 0 else off1
                        for c in range(nck):
                            j = ofs[b] + c
                            g, s = divmod(j, CPG)
                            m = get_mtile(l, cls, g)
                            nc.tensor.matmul(
                                pa[:],
                                lhsT=S[:, k * P:(k + 1) * P],
                                rhs=m[:, s * D:(s + 1) * D],
                                start=(k == 0), stop=(k == nch - 1))
                            k += 1
                    jj = junkp.tile([P, D], f32, name="junk", tag="junk")
                    nc.scalar.activation(jj[:], pa[:], AF.Square,
                                         accum_out=an2[:, b:b + 1])
                    # defer expmap scaling to the group epilogue
                    nc.scalar.copy(h_grid[:, tsl(b)], pa[:])
                    if (b + 1) % GRP == 0:
                        g0 = b + 1 - GRP
                        cs = slice(g0, b + 1)
                        sig = expmap_grid_cs("emB", an2, cs)
                        for t in range(g0, b + 1):
                            nc.vector.tensor_scalar_mul(
                                h_grid[:, tsl(t)], h_grid[:, tsl(t)],
                                sig[:, t:t + 1])
                        if l == 0:
                            for t in range(g0, b + 1):
                                emit_pass1(1, t)
                        else:
                            for t in range(g0, b + 1):
                                nc.sync.dma_start(
                                    out=out_ext[t * P:(t + 1) * P, :],
                                    in_=h_grid[:, tsl(t)])
                    # inject layer-1 stageA half 0 mid-phaseB(0) (Vector/
                    # Scalar work only; its AG trigger is emitted after the
                    # last gather of phaseB(0) so it never stalls the
                    # in-order GpSimd gather stream)
                    if (l == 0 and (b + 1) % GRP == 0
                            and b + 1 >= TH and b + 1 - GRP < TH):
                        emit_stageA(1, 0)
                if l == 0:
                    emit_AG(1, 0)

            # ================= emission =================
            # layer 0 phase A: per half, then its AllGather
            for hh in range(2):
                emit_init(hh)
                for t in range(hh * TH, (hh + 1) * TH):
                    emit_pass1(0, t)
                emit_stageA(0, hh)
                if hh == 1:
                    # keep the in-order GpSimd queue busy across the AG(0,1)
                    # trigger's input wait: class-0 gathers only need AG(0,0)
                    for _ in range(PRE):
                        issue_gather(0, 0)
                emit_AG(0, hh)
            # layer 0 phase B (embeds layer-1 pass1 + stageA half0; emits
            # AG(1,0) after its final gather)
            emit_phaseB(0)
            # layer 1 phase A second half + exchange
            emit_stageA(1, 1)
            for _ in range(PRE):
                issue_gather(1, 0)
            emit_AG(1, 1)
            # layer 1 phase B
            emit_phaseB(1)

    nc.compile()
    return nc


def _get_program(T, DC, c0, c1, NG0, NG1):
    key = (T, DC, tuple(c0), tuple(c1), NG0, NG1)
    if key not in _prog_cache:
        _prog_cache[key] = _build_program(T, DC, c0, c1, NG0, NG1)
    return _prog_cache[key]


# ----------------------------------------------------------------- entry

def run(inputs, trace=False, trace_kwargs=None):
    x = np.asarray(inputs["x"], np.float32)
    ei = np.asarray(inputs["edge_index"])
    W1 = np.asarray(inputs["W1"], np.float32)
    b1 = np.asarray(inputs["b1"], np.float32)
    W2 = np.asarray(inputs["W2"], np.float32)
    b2 = np.asarray(inputs["b2"], np.float32)
    N, D = x.shape
    assert D % P == 0
    meta, per_core = _host_prep(x, ei)
    T, DC = meta["T"], D // P
    c0, c1, NG0, NG1 = meta["c0"], meta["c1"], meta["NG0"], meta["NG1"]
    n_loc, perm = meta["n_loc"], meta["perm"]

    wt = np.stack([np.ascontiguousarray(W1.T), np.ascontiguousarray(W2.T)])
    wt = wt.astype(ml_dtypes.bfloat16)
    y = np.stack([np.tile(_np_expmap0(b1)[None, :], (P, 1)),
                  np.tile(_np_expmap0(b2)[None, :], (P, 1))])

    nc = _get_program(T, DC, c0, c1, NG0, NG1)
    in_maps = []
    for r in range(NCORES):
        m = dict(per_core[r])
        m["wt"] = wt
        m["y"] = y
        in_maps.append(m)

    kwargs = {}
    if trace:
        kwargs = dict(trace=True, trace_kwargs=trace_kwargs or {})
    res = run_bass_kernel_spmd(nc, in_maps, list(range(NCORES)), **kwargs)
    out = np.empty((N, D), np.float32)
    for r in range(NCORES):
        res_r = np.asarray(res.results[r]["out"])
        pr = perm[r]
        valid = pr >= 0
        out[r * n_loc + pr[valid]] = res_r[np.nonzero(valid)[0]]
    return out, res


def kernel(**inputs):
    out, _ = run(inputs)
    return out


# revision 23
# speedup vs baseline: 1.1106x; 1.0061x over previous
"""Trainium2 Bass kernel for the 2-layer hyperbolic (Poincare ball) GCN encoder.

Strategy (8 NeuronCores, SPMD):
  - Nodes sharded across cores (2500 rows/core, padded to 2560 = 20 tiles of 128),
    with a per-core degree-balanced permutation so every 128-destination block
    has ~equal edge count.
  - Weights replicated (bf16); dense mobius_matvec/mobius_add/logmap0 computed on
    the owned shard with all per-row reductions fused into per-partition scalar
    "grid" tensors of shape [128, T].
  - Per-layer exchange: tangent features (pre-scaled by deg^-0.5 on the source
    side) are AllGathered in bf16 across the 8 cores, SPLIT INTO TWO HALF-SHARD
    COLLECTIVES so each half is triggered as soon as its stageA tiles finish.
    A tiny warmup AllGather at kernel start absorbs the one-time CC-library
    load / mesh setup (~150us on the profiled baseline).
  - Edges partitioned by destination and CLASSIFIED BY SOURCE HALF: within each
    128-destination block, chunks whose sources live in half 0 are processed
    first, so phaseB's gathers begin while the half-1 AllGather is in flight.
    Per-block per-class chunk counts are ragged (baked at build time).
  - Messages fetched with dma_gather (1024 rows per instruction) round-robined
    over 4 SWDGE queues with rotating msg buffers; segment-sum on TensorE via
    0/1 selection matrices accumulated in PSUM.
  - Layer-1 phaseA is emitted inside layer-0 phaseB (pass1 in the group
    epilogues; stageA half 0 + its AllGather right after block T/2-1), so the
    layer-1 exchange is fully hidden behind layer-0's gather/matmul pipeline.
"""
import os
import numpy as np
import ml_dtypes

import concourse.bass as bass
import concourse.bacc as bacc
import concourse.tile as tile
import concourse.mybir as mybir
from concourse.bass_utils import run_bass_kernel_spmd
from concourse.masks import make_identity

NCORES = 8
P = 128
GN = 1024            # indices per dma_gather
CPG = GN // P        # chunks per gather
NQ = 4               # SWDGE queues
MN = 1.0 - 4e-3
EPS = 1e-15
ATEPS = 1e-7

f32 = mybir.dt.float32
bf16 = mybir.dt.bfloat16
i16 = mybir.dt.int16
AF = mybir.ActivationFunctionType
OP = mybir.AluOpType

_prog_cache = {}


# ----------------------------------------------------------------- host side

def _np_expmap0(u):
    u = np.asarray(u, np.float32)
    n = max(float(np.linalg.norm(u)), EPS)
    v = (np.tanh(n) * u / n).astype(np.float32)
    nn = max(float(np.linalg.norm(v)), EPS)
    if nn > MN:
        v = (v / nn * MN).astype(np.float32)
    return v


def _wrap_idx(lin, NGs):
    """[NGs*GN] linear slot order -> int16 [128, NGs*(GN//16)] wrapped."""
    w = lin.reshape(NGs, GN // 16, 16).transpose(2, 0, 1).reshape(16, -1)
    return np.tile(w.astype(np.int16), (8, 1))


def _host_prep(x, edge_index):
    x = np.asarray(x, np.float32)
    ei = np.asarray(edge_index)
    N, D = x.shape
    assert N % NCORES == 0
    n_loc = N // NCORES
    T = (n_loc + P - 1) // P
    assert T % 2 == 0
    TH = T // 2
    HR = TH * P                      # rows per half per core
    n_pad = T * P
    assert NCORES * HR <= 32767, "indices must fit int16"

    loops = np.arange(N, dtype=ei.dtype)
    ei = np.concatenate([ei, np.stack([loops, loops])], axis=1)
    row, col = ei[0].astype(np.int64), ei[1].astype(np.int64)
    deg = np.bincount(col, minlength=N).astype(np.float32)
    dis = (deg ** -0.5).astype(np.float32)

    # --- per-core degree-balanced slot permutation -------------------------
    inv_perm = np.empty((NCORES, n_loc), np.int64)   # local node -> slot
    perm = np.full((NCORES, n_pad), -1, np.int64)    # slot -> local node
    for r in range(NCORES):
        dloc = deg[r * n_loc:(r + 1) * n_loc]
        order = np.argsort(-dloc, kind="stable")
        ids = np.full(n_pad, -1, np.int64)
        ids[:n_loc] = order
        ids = ids.reshape(P, T)
        ids[1::2] = ids[1::2, ::-1]
        for b in range(T):
            blk_nodes = ids[:, b]
            for j, nd in enumerate(blk_nodes):
                slot = b * P + j
                perm[r, slot] = nd
                if nd >= 0:
                    inv_perm[r, nd] = slot

    src_core = row // n_loc
    src_slot = inv_perm[src_core, row % n_loc]
    src_half = src_slot // HR                         # 0 or 1
    src_row = src_core * HR + (src_slot % HR)         # row in ts_full[half]
    dst_core = col // n_loc
    dst_slot = inv_perm[dst_core, col % n_loc]

    blk = dst_slot // P + dst_core * T
    order = np.lexsort((src_row, src_half, blk))
    src_s = src_row[order]
    half_s = src_half[order]
    blk_s = blk[order]
    dstrel_s = (dst_slot[order] % P).astype(np.float32)

    blk_counts = np.bincount(blk_s, minlength=NCORES * T)
    bounds = np.concatenate([[0], np.cumsum(blk_counts)])

    # Ragged per-block per-class chunk layout. Identical across cores in
    # CHUNK COUNTS is required (one SPMD program) -> use the max per (b,cls)
    # over cores and pad. Slot streams are per-core.
    c_cnt = np.zeros((NCORES, T, 2), np.int64)
    for r in range(NCORES):
        for b in range(T):
            lo, hi = bounds[r * T + b], bounds[r * T + b + 1]
            h = half_s[lo:hi]
            n0 = int((h == 0).sum())
            n1 = int(hi - lo - n0)
            c_cnt[r, b, 0] = (n0 + P - 1) // P
            c_cnt[r, b, 1] = (n1 + P - 1) // P
    cmax = c_cnt.max(axis=0)                 # [T, 2] chunks per block/class
    c0 = [int(v) for v in cmax[:, 0]]
    c1 = [int(v) for v in cmax[:, 1]]
    tot0, tot1 = sum(c0), sum(c1)
    NG0 = (tot0 * P + GN - 1) // GN
    NG1 = (tot1 * P + GN - 1) // GN

    # per-core slot streams (gather indices + dst-rel for S build)
    gl0 = np.zeros((NCORES, NG0 * GN), np.int64)
    gl1 = np.zeros((NCORES, NG1 * GN), np.int64)
    edst = np.full((NCORES, P, tot0 + tot1), -1.0, np.float32)
    off0 = np.concatenate([[0], np.cumsum(c0)])
    off1 = np.concatenate([[0], np.cumsum(c1)])
    colof = np.concatenate([[0], np.cumsum([a + b for a, b in zip(c0, c1)])])
    for r in range(NCORES):
        for b in range(T):
            lo, hi = bounds[r * T + b], bounds[r * T + b + 1]
            h = half_s[lo:hi]
            for cls, (glin, offs) in enumerate(((gl0, off0), (gl1, off1))):
                sel = np.nonzero(h == cls)[0] + lo
                L = len(sel)
                nch = (c0[b] if cls == 0 else c1[b])
                pad = np.zeros(nch * P, np.int64)
                pad[:L] = src_s[sel]
                glin[r, offs[b] * P:(offs[b] + nch) * P] = pad
                dpad = np.full(nch * P, -1.0, np.float32)
                dpad[:L] = dstrel_s[sel]
                cbase = colof[b] + (0 if cls == 0 else c0[b])
                edst[r][:, cbase:cbase + nch] = dpad.reshape(nch, P).T

    idx0 = np.stack([_wrap_idx(gl0[r], NG0) for r in range(NCORES)])
    idx1 = np.stack([_wrap_idx(gl1[r], NG1) for r in range(NCORES)])

    dis_loc = np.zeros((NCORES, P, T), np.float32)
    x_loc = np.zeros((NCORES, n_pad, D), np.float32)
    for r in range(NCORES):
        for slot in range(n_pad):
            nd = perm[r, slot]
            if nd >= 0:
                dis_loc[r, slot % P, slot // P] = dis[r * n_loc + nd]
                x_loc[r, slot] = x[r * n_loc + nd]

    iota = np.tile(np.arange(P, dtype=np.float32)[None, :], (P, 1))
    meta = dict(N=N, D=D, n_loc=n_loc, T=T, n_pad=n_pad, perm=perm,
                c0=c0, c1=c1, NG0=NG0, NG1=NG1)
    per_core = [dict(x=x_loc[r].astype(ml_dtypes.bfloat16),
                     dis=dis_loc[r],
                     gidx0=idx0[r], gidx1=idx1[r],
                     edst=edst[r].astype(ml_dtypes.bfloat16),
                     iota=iota.astype(ml_dtypes.bfloat16))
                for r in range(NCORES)]
    return meta, per_core


# --------------------------------------------------------------- device side

def _build_program(T, DC, c0, c1, NG0, NG1):
    D = DC * P
    TH = T // 2
    HR = TH * P
    NPAD = T * P
    EX = bf16
    TOT = sum(c0) + sum(c1)
    CMAX = max(a + b for a, b in zip(c0, c1))
    GRP = int(os.environ.get("KGRP", "5"))
    assert (T // 2) % GRP == 0, "group size must divide half the tiles"
    MSGB = int(os.environ.get("KMSGB", "8"))
    PRE = int(os.environ.get("KPRE", "4"))

    nc = bacc.Bacc("TRN2", target_bir_lowering=False, debug=False,
                   num_devices=NCORES, num_swdge_queues=NQ,
                   dynamic_dma_scratch_size=int(os.environ.get("KSCRATCH",
                                                               "16384")))

    x_in = nc.dram_tensor("x", [NPAD, D], bf16, kind="ExternalInput")
    wt_in = nc.dram_tensor("wt", [2, D, D], bf16, kind="ExternalInput")
    y_in = nc.dram_tensor("y", [2, P, D], f32, kind="ExternalInput")
    iota_in = nc.dram_tensor("iota", [P, P], bf16, kind="ExternalInput")
    dis_in = nc.dram_tensor("dis", [P, T], f32, kind="ExternalInput")
    g0_in = nc.dram_tensor("gidx0", [P, NG0 * (GN // 16)], i16,
                           kind="ExternalInput")
    g1_in = nc.dram_tensor("gidx1", [P, NG1 * (GN // 16)], i16,
                           kind="ExternalInput")
    edst_in = nc.dram_tensor("edst", [P, TOT], bf16, kind="ExternalInput")
    out_ext = nc.dram_tensor("out", [NPAD, D], f32, kind="ExternalOutput")

    with tile.TileContext(nc) as tc:
        with (
            tc.tile_pool(name="const", bufs=1) as constp,
            tc.tile_pool(name="grid", bufs=1) as gridp,
            tc.tile_pool(name="big", bufs=1) as bigp,
            tc.tile_pool(name="work", bufs=3) as workp,
            tc.tile_pool(name="junk", bufs=3) as junkp,
            tc.tile_pool(name="msgs", bufs=MSGB) as msgp,
            tc.tile_pool(name="sblk", bufs=2) as sblkp,
            tc.tile_pool(name="psum", bufs=1, space="PSUM") as psump,
            tc.tile_pool(name="psag", bufs=GRP, space="PSUM") as psagp,
            tc.tile_pool(name="dram", bufs=1, space="DRAM") as dramp,
        ):
            # ---- CC warmup: tiny AllGather issued before anything else ----
            warm_l = dramp.tile([P, 16], bf16, name="warm_l", tag="warm_l")
            warm_g = dramp.tile([NCORES * P, 16], bf16, addr_space="Shared",
                                name="warm_g", tag="warm_g")
            nc.sync.dma_start(out=warm_l[:], in_=iota_in[:, 0:16])
            nc.gpsimd.collective_compute(
                "AllGather", OP.bypass,
                replica_groups=[list(range(NCORES))],
                ins=[warm_l.opt()], outs=[warm_g.opt()])

            # ---- constants ----
            wt_sb = constp.tile([P, 2 * DC * D], bf16, name="wt", tag="wt")
            for l in range(2):
                for k in range(DC):
                    nc.sync.dma_start(
                        out=wt_sb[:, (l * DC + k) * D:(l * DC + k + 1) * D],
                        in_=wt_in[l, k * P:(k + 1) * P, :])
            y_sb = constp.tile([P, 2 * D], f32, name="y", tag="y")
            nc.sync.dma_start(out=y_sb[:, 0:D], in_=y_in[0])
            nc.sync.dma_start(out=y_sb[:, D:2 * D], in_=y_in[1])
            iota_sb = constp.tile([P, P], bf16, name="iota", tag="iota")
            nc.sync.dma_start(out=iota_sb[:], in_=iota_in[:, :])
            ident = constp.tile([P, P], f32, name="ident", tag="ident")
            make_identity(nc, ident[:])
            disg = constp.tile([P, T], f32, name="dis", tag="dis")
            nc.sync.dma_start(out=disg[:], in_=dis_in[:, :])
            g0_sb = constp.tile([P, NG0 * (GN // 16)], i16, name="g0",
                                tag="g0")
            nc.sync.dma_start(out=g0_sb[:], in_=g0_in[:, :])
            g1_sb = constp.tile([P, NG1 * (GN // 16)], i16, name="g1",
                                tag="g1")
            nc.sync.dma_start(out=g1_sb[:], in_=g1_in[:, :])
            edst_sb = constp.tile([P, TOT], bf16, name="edst", tag="edst")
            nc.sync.dma_start(out=edst_sb[:], in_=edst_in[:, :])

            # ---- persistent big tensors ----
            h_grid = bigp.tile([P, T * D], f32, name="h", tag="h")  # h then u
            agg_grid = bigp.tile([P, T * D], bf16, name="agg", tag="agg")
            xstage = bigp.tile([P, TH * D], bf16, name="xs", tag="xs")
            hn2 = gridp.tile([P, T], f32, name="hn2", tag="hn2")

            def G(tag):
                return gridp.tile([P, T], f32, name=tag, tag=tag)

            def tsl(t):
                return slice(t * D, (t + 1) * D)

            mxn2_g = [G("mxn2_0"), G("mxn2_1")]

            def emit_pass1(l, t):
                pt = psump.tile([P, D], f32, name="pt", tag="pt")
                for k in range(DC):
                    nc.tensor.transpose(
                        out=pt[:, k * P:(k + 1) * P],
                        in_=h_grid[:, t * D + k * P: t * D + (k + 1) * P],
                        identity=ident[:])
                hT = workp.tile([P, D], bf16, name="hT", tag="hT")
                nc.scalar.copy(hT[:], pt[:])
                pm = psump.tile([P, D], f32, name="pm", tag="pm")
                for k in range(DC):
                    nc.tensor.matmul(
                        pm[:],
                        lhsT=hT[:, k * P:(k + 1) * P],
                        rhs=wt_sb[:, (l * DC + k) * D:(l * DC + k + 1) * D],
                        start=(k == 0), stop=(k == DC - 1))
                nc.scalar.copy(agg_grid[:, tsl(t)], pm[:])
                jj = junkp.tile([P, D], f32, name="junk", tag="junk")
                nc.scalar.activation(jj[:], pm[:], AF.Square,
                                     accum_out=mxn2_g[l][:, t:t + 1])

            # ---- exchange tensors: two halves per layer ----
            ts_loc = [[dramp.tile([HR, D], EX, name=f"ts_loc{l}_{h}",
                                  tag=f"ts_loc{l}_{h}") for h in range(2)]
                      for l in range(2)]
            ts_full = [[dramp.tile([NCORES * HR, D], EX, addr_space="Shared",
                                   name=f"ts_full{l}_{h}",
                                   tag=f"ts_full{l}_{h}") for h in range(2)]
                       for l in range(2)]
            y2col = gridp.tile([P, 1], f32, name="y2col", tag="y2col")
            avt = {}

            def GA(tag):
                if tag not in avt:
                    avt[tag] = G(tag)
                return avt[tag]

            def artanh2h(nm, xx, cs):
                xcl = GA(nm + "_xcl")
                nc.vector.tensor_scalar_min(xcl[:, cs], xx[:, cs],
                                            1.0 - ATEPS)
                a1 = GA(nm + "_a1")
                nc.scalar.activation(a1[:, cs], xcl[:, cs], AF.Ln,
                                     bias=1.0, scale=1.0)
                omx = GA(nm + "_omx")
                nc.vector.tensor_scalar(out=omx[:, cs], in0=xcl[:, cs],
                                        scalar1=-1.0, scalar2=1.0,
                                        op0=OP.mult, op1=OP.add)
                a2 = GA(nm + "_a2")
                nc.scalar.activation(a2[:, cs], omx[:, cs], AF.Ln)
                at2 = GA(nm + "_at2")
                nc.vector.tensor_tensor(out=at2[:, cs], in0=a1[:, cs],
                                        in1=a2[:, cs], op=OP.subtract)
                return at2

            def emit_stageA(l, cs, trng, first):
                """mobius_add scalar stages + passes 2/3 + ts out for tiles
                trng (columns cs) of layer l."""
                y_ap = y_sb[:, l * D:(l + 1) * D]
                mxn2 = mxn2_g[l]
                if first:
                    jy = junkp.tile([P, D], f32, name="junk", tag="junk")
                    nc.scalar.activation(jy[:], y_ap, AF.Square,
                                         accum_out=y2col[:])
                # stage 1
                xn = GA("xn")
                nc.scalar.activation(xn[:, cs], hn2[:, cs], AF.Sqrt)
                mxn = GA("mxn")
                nc.scalar.activation(mxn[:, cs], mxn2[:, cs], AF.Sqrt)
                xng = GA("xng")
                nc.vector.tensor_scalar_max(xng[:, cs], xn[:, cs], EPS)
                xrec = GA("xrec")
                nc.vector.reciprocal(xrec[:, cs], xng[:, cs])
                at2 = artanh2h("s1", xn, cs)
                rr2 = GA("rr2")
                nc.vector.tensor_tensor(out=rr2[:, cs], in0=at2[:, cs],
                                        in1=xrec[:, cs], op=OP.mult)
                mxng = GA("mxng")
                nc.vector.tensor_scalar_max(mxng[:, cs], mxn[:, cs], EPS)
                mrec = GA("mrec")
                nc.vector.reciprocal(mrec[:, cs], mxng[:, cs])
                cc = GA("cc")
                nc.vector.scalar_tensor_tensor(out=cc[:, cs],
                                               in0=mxn[:, cs], scalar=0.5,
                                               in1=rr2[:, cs],
                                               op0=OP.mult, op1=OP.mult)
                tch = GA("tch")
                nc.scalar.activation(tch[:, cs], cc[:, cs], AF.Tanh)
                tcg = GA("tcg")
                nc.vector.tensor_scalar_max(tcg[:, cs], tch[:, cs], EPS)
                tcrec = GA("tcrec")
                nc.vector.reciprocal(tcrec[:, cs], tcg[:, cs])
                psA = GA("psA")
                nc.vector.tensor_scalar(out=psA[:, cs], in0=tcrec[:, cs],
                                        scalar1=MN, scalar2=1.0,
                                        op0=OP.mult, op1=OP.min)
                sp0 = GA("sp0")
                nc.vector.tensor_tensor(out=sp0[:, cs], in0=tch[:, cs],
                                        in1=mrec[:, cs], op=OP.mult)
                spg = GA("spg")
                nc.vector.tensor_tensor(out=spg[:, cs], in0=sp0[:, cs],
                                        in1=psA[:, cs], op=OP.mult)
                tcm = GA("tcm")
                nc.vector.tensor_scalar_min(tcm[:, cs], tch[:, cs], MN)
                x2 = GA("x2")
                nc.vector.tensor_tensor(out=x2[:, cs], in0=tcm[:, cs],
                                        in1=tcm[:, cs], op=OP.mult)
                # pass 2: xy = sum((sp*mx) . y)
                xy = GA("xy")
                for t in trng:
                    jx = junkp.tile([P, D], f32, name="junk", tag="junk")
                    nc.vector.scalar_tensor_tensor(
                        out=jx[:], in0=agg_grid[:, tsl(t)],
                        scalar=spg[:, t:t + 1], in1=y_ap,
                        op0=OP.mult, op1=OP.mult,
                        accum_out=xy[:, t:t + 1])
                # stage 2
                t0 = GA("t0")
                nc.vector.tensor_scalar(out=t0[:, cs], in0=xy[:, cs],
                                        scalar1=2.0, scalar2=1.0,
                                        op0=OP.mult, op1=OP.add)
                ag = GA("ag")
                nc.vector.tensor_scalar_add(ag[:, cs], t0[:, cs],
                                            y2col[:, 0:1])
                d0 = GA("d0")
                nc.vector.tensor_scalar_mul(d0[:, cs], x2[:, cs],
                                            y2col[:, 0:1])
                d1 = GA("d1")
                nc.vector.tensor_tensor(out=d1[:, cs], in0=d0[:, cs],
                                        in1=t0[:, cs], op=OP.add)
                dg = GA("dg")
                nc.vector.tensor_scalar_max(dg[:, cs], d1[:, cs], EPS)
                dinv = GA("dinv")
                nc.vector.reciprocal(dinv[:, cs], dg[:, cs])
                alpha = GA("alpha")
                nc.vector.tensor_tensor(out=alpha[:, cs], in0=ag[:, cs],
                                        in1=dinv[:, cs], op=OP.mult)
                bsc = GA("bsc")
                nc.vector.tensor_scalar(out=bsc[:, cs], in0=x2[:, cs],
                                        scalar1=-1.0, scalar2=1.0,
                                        op0=OP.mult, op1=OP.add)
                beta = GA("beta")
                nc.vector.tensor_tensor(out=beta[:, cs], in0=bsc[:, cs],
                                        in1=dinv[:, cs], op=OP.mult)
                alphasp = GA("alphasp")
                nc.vector.tensor_tensor(out=alphasp[:, cs],
                                        in0=alpha[:, cs], in1=spg[:, cs],
                                        op=OP.mult)
                # pass 3: u = alphasp*mx + beta*y (into h_grid)
                un2 = GA("un2")
                for t in trng:
                    t1 = workp.tile([P, D], f32, name="t1", tag="t1")
                    nc.vector.tensor_scalar_mul(t1[:], y_ap,
                                                beta[:, t:t + 1])
                    us = h_grid[:, tsl(t)]
                    nc.vector.scalar_tensor_tensor(
                        out=us, in0=agg_grid[:, tsl(t)],
                        scalar=alphasp[:, t:t + 1], in1=t1[:],
                        op0=OP.mult, op1=OP.add)
                    ju = junkp.tile([P, D], f32, name="junk", tag="junk")
                    nc.scalar.activation(ju[:], us, AF.Square,
                                         accum_out=un2[:, t:t + 1])
                # stage 3: gamma
                un = GA("un")
                nc.scalar.activation(un[:, cs], un2[:, cs], AF.Sqrt)
                ung = GA("ung")
                nc.vector.tensor_scalar_max(ung[:, cs], un[:, cs], EPS)
                urec = GA("urec")
                nc.vector.reciprocal(urec[:, cs], ung[:, cs])
                h2n = GA("h2n")
                nc.vector.tensor_scalar_min(h2n[:, cs], un[:, cs], MN)
                at2u = artanh2h("s3", h2n, cs)
                h2ng = GA("h2ng")
                nc.vector.tensor_scalar_max(h2ng[:, cs], h2n[:, cs], EPS)
                hrec = GA("hrec")
                nc.vector.reciprocal(hrec[:, cs], h2ng[:, cs])
                lam2 = GA("lam2")
                nc.vector.tensor_tensor(out=lam2[:, cs], in0=at2u[:, cs],
                                        in1=hrec[:, cs], op=OP.mult)
                pst = GA("pst")
                nc.vector.tensor_scalar(out=pst[:, cs], in0=urec[:, cs],
                                        scalar1=MN, scalar2=1.0,
                                        op0=OP.mult, op1=OP.min)
                gm0 = GA("gm0")
                nc.vector.scalar_tensor_tensor(out=gm0[:, cs],
                                               in0=lam2[:, cs], scalar=0.5,
                                               in1=pst[:, cs],
                                               op0=OP.mult, op1=OP.mult)
                gam = GA("gam")
                nc.vector.tensor_tensor(out=gam[:, cs], in0=gm0[:, cs],
                                        in1=disg[:, cs], op=OP.mult)
                # ts tiles out (ScalarE: copy with per-partition scale)
                for t in trng:
                    tst = workp.tile([P, D], EX, name="tst", tag="tst")
                    nc.scalar.activation(tst[:], h_grid[:, tsl(t)],
                                         AF.Copy, scale=gam[:, t:t + 1])
                    hh = t // TH
                    t_rel = t - hh * TH
                    nc.sync.dma_start(
                        out=ts_loc[l][hh][t_rel * P:(t_rel + 1) * P, :],
                        in_=tst[:])

            def emit_AG(l, hh):
                nc.gpsimd.collective_compute(
                    "AllGather", OP.bypass,
                    replica_groups=[list(range(NCORES))],
                    ins=[ts_loc[l][hh].opt()], outs=[ts_full[l][hh].opt()])

            bvt = {}

            def GB(tag):
                if tag not in bvt:
                    bvt[tag] = G(tag)
                return bvt[tag]

            def expmap_grid_cs(nm, n2, cs, with_dis=True):
                """sig2 columns cs of expmap0(dis*agg) incl. dst-side dis
                (or plain expmap0 scaling when with_dis=False);
                also writes hn2[:, cs]."""
                n = GB(nm + "_n")
                nc.scalar.activation(n[:, cs], n2[:, cs], AF.Sqrt)
                if with_dis:
                    npr = GB(nm + "_npr")
                    nc.vector.tensor_tensor(out=npr[:, cs], in0=n[:, cs],
                                            in1=disg[:, cs], op=OP.mult)
                else:
                    npr = n
                ng = GB(nm + "_ng")
                nc.vector.tensor_scalar_max(ng[:, cs], npr[:, cs], EPS)
                tn = GB(nm + "_tn")
                nc.scalar.activation(tn[:, cs], npr[:, cs], AF.Tanh)
                rec = GB(nm + "_rec")
                nc.vector.reciprocal(rec[:, cs], ng[:, cs])
                sc0 = GB(nm + "_sc0")
                nc.vector.tensor_tensor(out=sc0[:, cs], in0=tn[:, cs],
                                        in1=rec[:, cs], op=OP.mult)
                tng = GB(nm + "_tng")
                nc.vector.tensor_scalar_max(tng[:, cs], tn[:, cs], EPS)
                trec = GB(nm + "_trec")
                nc.vector.reciprocal(trec[:, cs], tng[:, cs])
                ps = GB(nm + "_ps")
                nc.vector.tensor_scalar(out=ps[:, cs], in0=trec[:, cs],
                                        scalar1=MN, scalar2=1.0,
                                        op0=OP.mult, op1=OP.min)
                sig = GB(nm + "_sig")
                nc.vector.tensor_tensor(out=sig[:, cs], in0=sc0[:, cs],
                                        in1=ps[:, cs], op=OP.mult)
                if with_dis:
                    sig2 = GB(nm + "_sig2")
                    nc.vector.tensor_tensor(out=sig2[:, cs], in0=sig[:, cs],
                                            in1=disg[:, cs], op=OP.mult)
                    sig = sig2
                tnm = GB(nm + "_tnm")
                nc.vector.tensor_scalar_min(tnm[:, cs], tn[:, cs], MN)
                nc.vector.tensor_tensor(out=hn2[:, cs], in0=tnm[:, cs],
                                        in1=tnm[:, cs], op=OP.mult)
                return sig

            # ---- init: h = expmap0(x), one half at a time ----
            n2i = G("n2i")

            def emit_init(hh):
                cs = slice(hh * TH, (hh + 1) * TH)
                for t in range(hh * TH, (hh + 1) * TH):
                    t_rel = t - hh * TH
                    xs = xstage[:, t_rel * D:(t_rel + 1) * D]
                    nc.sync.dma_start(out=xs, in_=x_in[t * P:(t + 1) * P, :])
                    jj = junkp.tile([P, D], f32, name="junk", tag="junk")
                    nc.scalar.activation(jj[:], xs, AF.Square,
                                         accum_out=n2i[:, t:t + 1])
                sig0 = expmap_grid_cs("em0", n2i, cs, with_dis=False)
                for t in range(hh * TH, (hh + 1) * TH):
                    t_rel = t - hh * TH
                    xs = xstage[:, t_rel * D:(t_rel + 1) * D]
                    nc.vector.tensor_scalar_mul(h_grid[:, tsl(t)], xs,
                                                sig0[:, t:t + 1])

            # ---- phaseB ----
            # chunk -> gather bookkeeping (per class stream)
            off0 = [0]
            off1 = [0]
            for b in range(T):
                off0.append(off0[-1] + c0[b])
                off1.append(off1[-1] + c1[b])
            colof = [0]
            for b in range(T):
                colof.append(colof[-1] + c0[b] + c1[b])
            gsb = [g0_sb, g1_sb]
            NGs = [NG0, NG1]
            qctr = [0]
            g_issued = {}   # (l, cls) -> number of gathers issued
            g_avail = {}    # (l, cls) -> {g: msg tile}

            def issue_gather(l, cls):
                g = g_issued.get((l, cls), 0)
                if g >= NGs[cls]:
                    return
                m = msgp.tile([P, CPG * D], EX, name="m", tag="m")
                nc.gpsimd.dma_gather(
                    m[:].rearrange("p (c e) -> p c e", c=CPG),
                    ts_full[l][cls],
                    gsb[cls][:, g * (GN // 16):(g + 1) * (GN // 16)],
                    GN, GN, D, queue_num=qctr[0] % NQ)
                qctr[0] += 1
                g_issued[(l, cls)] = g + 1
                g_avail.setdefault((l, cls), {})[g] = m

            def get_mtile(l, cls, g):
                while g_issued.get((l, cls), 0) <= g:
                    issue_gather(l, cls)
                av = g_avail[(l, cls)]
                for gg in [k for k in av if k < g]:
                    del av[gg]       # consumed; release reference
                return av[g]

            # static gather consumption order for issue-ahead
            def consume_order(l):
                seq = []
                seen = set()
                for b in range(T):
                    for cls in range(2):
                        nck = c0[b] if cls == 0 else c1[b]
                        ofs = off0 if cls == 0 else off1
                        for c in range(nck):
                            g = (ofs[b] + c) // CPG
                            if (cls, g) not in seen:
                                seen.add((cls, g))
                                seq.append((cls, g))
                # trailing pad gathers (never consumed) appended last
                for cls, ngs in ((0, NG0), (1, NG1)):
                    for g in range(ngs):
                        if (cls, g) not in seen:
                            seq.append((cls, g))
                return seq

            def issue_ahead(l, n, only_cls=None):
                """Issue the next n not-yet-issued gathers of layer l in
                consumption order (keeps the in-order GpSimd queue busy
                across a collective trigger's input wait). only_cls
                restricts to one class stream: a gather reading an
                AllGather output whose trigger is emitted LATER would
                deadlock the in-order GpSimd queue."""
                done = 0
                for cls, g in consume_order(l):
                    if done >= n:
                        break
                    if only_cls is not None and cls != only_cls:
                        continue
                    if g >= g_issued.get((l, cls), 0):
                        issue_gather(l, cls)
                        done += 1

            def emit_phaseB(l):
                an2 = G("an2")
                pa_live = {}
                for b in range(T):
                    nch = c0[b] + c1[b]
                    S = sblkp.tile([P, CMAX * P], EX, name="S", tag="S")
                    nc.vector.tensor_tensor(
                        out=S[:, 0:nch * P].rearrange(
                            "p (c j) -> p c j", c=nch),
                        in0=edst_sb[:, colof[b]:colof[b] + nch].to_broadcast(
                            [P, nch, P]),
                        in1=iota_sb[:].rearrange("p (o j) -> p o j", o=1)
                            .to_broadcast([P, nch, P]),
                        op=OP.is_equal)
                    pa = psagp.tile([P, D], f32, name="pa", tag="pa")
                    pa_live[b] = pa
                    k = 0
                    for cls in range(2):
                        nck = c0[b] if cls == 0 else c1[b]
                        ofs = off0 if cls == 0 else off1
                        for c in range(nck):
                            j = ofs[b] + c
                            g, s = divmod(j, CPG)
                            m = get_mtile(l, cls, g)
                            nc.tensor.matmul(
                                pa[:],
                                lhsT=S[:, k * P:(k + 1) * P],
                                rhs=m[:, s * D:(s + 1) * D],
                                start=(k == 0), stop=(k == nch - 1))
                            k += 1
                    jj = junkp.tile([P, D], f32, name="junk", tag="junk")
                    nc.scalar.activation(jj[:], pa[:], AF.Square,
                                         accum_out=an2[:, b:b + 1])
                    # expmap scaling deferred to the group epilogue; pa stays
                    # in PSUM until then (fused copy+scale on ScalarE)
                    if (b + 1) % GRP == 0:
                        g0 = b + 1 - GRP
                        cs = slice(g0, b + 1)
                        sig = expmap_grid_cs("emB", an2, cs)
                        for t in range(g0, b + 1):
                            nc.scalar.activation(
                                h_grid[:, tsl(t)], pa_live.pop(t)[:],
                                AF.Copy, scale=sig[:, t:t + 1])
                        if l == 0:
                            for t in range(g0, b + 1):
                                emit_pass1(1, t)
                            # layer-1 stageA sliced per group: Vector work
                            # interleaves with later blocks' S builds
                            emit_stageA(1, cs, range(g0, b + 1),
                                        first=(g0 == 0))
                            if b + 1 == TH:
                                issue_ahead(0, 4)
                                emit_AG(1, 0)
                            if b + 1 == T:
                                issue_ahead(1, 4, only_cls=0)
                                emit_AG(1, 1)
                        else:
                            for t in range(g0, b + 1):
                                nc.sync.dma_start(
                                    out=out_ext[t * P:(t + 1) * P, :],
                                    in_=h_grid[:, tsl(t)])

            # ================= emission =================
            # layer 0 phase A: per half, then its AllGather
            for hh in range(2):
                emit_init(hh)
                for t in range(hh * TH, (hh + 1) * TH):
                    emit_pass1(0, t)
                emit_stageA(0, slice(hh * TH, (hh + 1) * TH),
                            range(hh * TH, (hh + 1) * TH), first=(hh == 0))
                if hh == 1:
                    # keep the in-order GpSimd queue busy across the AG(0,1)
                    # trigger's input wait: class-0 gathers only need AG(0,0)
                    issue_ahead(0, PRE, only_cls=0)
                emit_AG(0, hh)
            # layer 0 phase B (embeds all of layer-1 phase A, sliced into the
            # group epilogues, plus both layer-1 AllGather triggers)
            emit_phaseB(0)
            # layer 1 phase B
            emit_phaseB(1)

    nc.compile()
    return nc


def _get_program(T, DC, c0, c1, NG0, NG1):
    key = (T, DC, tuple(c0), tuple(c1), NG0, NG1)
    if key not in _prog_cache:
        _prog_cache[key] = _build_program(T, DC, c0, c1, NG0, NG1)
    return _prog_cache[key]


# ----------------------------------------------------------------- entry

def run(inputs, trace=False, trace_kwargs=None):
    x = np.asarray(inputs["x"], np.float32)
    ei = np.asarray(inputs["edge_index"])
    W1 = np.asarray(inputs["W1"], np.float32)
    b1 = np.asarray(inputs["b1"], np.float32)
    W2 = np.asarray(inputs["W2"], np.float32)
    b2 = np.asarray(inputs["b2"], np.float32)
    N, D = x.shape
    assert D % P == 0
    meta, per_core = _host_prep(x, ei)
    T, DC = meta["T"], D // P
    c0, c1, NG0, NG1 = meta["c0"], meta["c1"], meta["NG0"], meta["NG1"]
    n_loc, perm = meta["n_loc"], meta["perm"]

    wt = np.stack([np.ascontiguousarray(W1.T), np.ascontiguousarray(W2.T)])
    wt = wt.astype(ml_dtypes.bfloat16)
    y = np.stack([np.tile(_np_expmap0(b1)[None, :], (P, 1)),
                  np.tile(_np_expmap0(b2)[None, :], (P, 1))])

    nc = _get_program(T, DC, c0, c1, NG0, NG1)
    in_maps = []
    for r in range(NCORES):
        m = dict(per_core[r])
        m["wt"] = wt
        m["y"] = y
        in_maps.append(m)

    kwargs = {}
    if trace:
        kwargs = dict(trace=True, trace_kwargs=trace_kwargs or {})
    res = run_bass_kernel_spmd(nc, in_maps, list(range(NCORES)), **kwargs)
    out = np.empty((N, D), np.float32)
    for r in range(NCORES):
        res_r = np.asarray(res.results[r]["out"])
        pr = perm[r]
        valid = pr >= 0
        out[r * n_loc + pr[valid]] = res_r[np.nonzero(valid)[0]]
    return out, res


def kernel(**inputs):
    out, _ = run(inputs)
    return out
